# revision 1
# baseline (speedup 1.0000x reference)
"""Cross-attention kernel for 8 Trainium2 NeuronCores (Bass/Tile).

Sharding: data-parallel over (L, query-half). Core c handles batch l = c//2
and queries [(c%2)*1024, (c%2+1)*1024) of that batch. K/V for the full 2048
keys of batch l are computed on both cores of the pair (duplicated work, no
cross-core communication).

Per-core dataflow (matmuls in bf16 with f32 PSUM accumulation):
  qT[e, nq], kT[e, nk]   transposed projections (feature dim on partitions)
  v'[nk, h, 65]          v projection + a ones column per head (softmax denom)
  sT[nk, nq] = kT.T @ qT per head  -- scores transposed, keys on partitions;
                                      head pairs run row-packed on the PE
  attnT = exp(SCALE*sT + maskbias[nk])  one ACT op fuses scale+mask+exp+cast
  out'[65, nq] = v'.T @ attnT      rows 0..63: head out^T, row 64: denominator
  normalize: DVE reciprocal of row 64, broadcast across partitions via a
             tiny ones-block matmul, DVE multiply
  out = out_allT.T @ WoT + bo      final projection, bias via a K=1 matmul
"""

import numpy as np
import ml_dtypes
from contextlib import ExitStack

import concourse.bass as bass
import concourse.tile as tile
from concourse import bacc, mybir
from concourse.bass_utils import run_bass_kernel_spmd

L, N, D_IN = 4, 2048, 1024
H, DH = 8, 64
INNER = H * DH          # 512
D_OUT = D_IN
SCALE = DH ** -0.5      # 0.125
NQ = N // 2             # 1024 queries per core
NCORES = 8
DC = D_IN // 128        # 8 contraction chunks for the projections
EC = INNER // 128       # 4 feature chunks (= head pairs)
KC = N // 128           # 16 key chunks
NKB = N // 512          # 4 key 512-blocks
NQB = NQ // 512         # 2 query 512-blocks
MASK_NEG = -50.0

BF = mybir.dt.bfloat16
F32 = mybir.dt.float32
EXP = mybir.ActivationFunctionType.Exp


def _emit(ctx, tc, xT, wqT, wkT, wvT, woT, bo, maskb, out):
    nc = tc.nc

    const = ctx.enter_context(tc.tile_pool(name="const", bufs=1))
    big = ctx.enter_context(tc.tile_pool(name="big", bufs=1))
    attn_sb = ctx.enter_context(tc.tile_pool(name="attn_sb", bufs=4))
    norm_sb = ctx.enter_context(tc.tile_pool(name="norm_sb", bufs=3))
    stage_sb = ctx.enter_context(tc.tile_pool(name="stage_sb", bufs=3))
    out_sb = ctx.enter_context(tc.tile_pool(name="out_sb", bufs=2))
    ps_st = ctx.enter_context(tc.tile_pool(name="ps_st", bufs=2, space="PSUM"))
    ps_av = ctx.enter_context(tc.tile_pool(name="ps_av", bufs=4, space="PSUM"))

    # ---- inputs -> SBUF. Weight DRAM layouts are partition-major so each
    # loads in one DMA with fat (8KB) per-partition lines; x transposed
    # loads per (d-chunk, key-half) with 2KB lines. Each core's x arrives
    # with its own query half permuted to keys 0..1023 (softmax and AV are
    # permutation-invariant over keys; the mask bias is permuted to match),
    # so the q projection just reads the first NQ columns of xT.
    wk_s = const.tile([128, DC, INNER], BF)
    wq_s = const.tile([128, DC, INNER], BF)
    wv_s = const.tile([128, DC, INNER], BF)
    wo_s = const.tile([128, EC, D_OUT], BF)
    bo_s = const.tile([1, D_OUT], F32)
    maskb_s = const.tile([128, KC], F32)
    xT_s = big.tile([128, DC, N], BF)
    nc.sync.dma_start(wk_s, wkT)
    for d in range(DC):
        nc.sync.dma_start(xT_s[:, d, 0:NQ], xT[d][:, 0:NQ])
    nc.sync.dma_start(wq_s, wqT)
    nc.sync.dma_start(maskb_s, maskb)
    for d in range(DC):
        nc.sync.dma_start(xT_s[:, d, NQ:N], xT[d][:, NQ:N])
    nc.sync.dma_start(wv_s, wvT)
    nc.sync.dma_start(wo_s, woT)
    nc.sync.dma_start(bo_s, bo)

    ones_row = const.tile([1, 128], BF)
    nc.vector.memset(ones_row, 1.0)
    ones33 = const.tile([33, 128], BF)
    nc.vector.memset(ones33, 1.0)
    bo_bf = const.tile([1, D_OUT], BF)
    nc.vector.tensor_copy(bo_bf, bo_s)

    kT_s = big.tile([128, EC, N], BF)
    qT_s = big.tile([128, EC, NQ], BF)
    vp_s = big.tile([128, KC, H, DH + 1], BF)
    nc.vector.memset(vp_s[:, :, :, DH], 1.0)
    out_allT = big.tile([128, EC, NQ], BF)

    def proj_kT(j, b):
        ps = ps_av.tile([128, 512], F32, tag="av", name=f"ps_k{j}{b}")
        for d in range(DC):
            nc.tensor.matmul(
                ps, wk_s[:, d, j * 128:(j + 1) * 128],
                xT_s[:, d, b * 512:(b + 1) * 512],
                start=(d == 0), stop=(d == DC - 1))
        nc.vector.tensor_copy(kT_s[:, j, b * 512:(b + 1) * 512], ps)

    def proj_qT(j, b):
        ps = ps_av.tile([128, 512], F32, tag="av", name=f"ps_q{j}{b}")
        for d in range(DC):
            nc.tensor.matmul(
                ps, wq_s[:, d, j * 128:(j + 1) * 128],
                xT_s[:, d, b * 512:(b + 1) * 512],
                start=(d == 0), stop=(d == DC - 1))
        nc.vector.tensor_copy(qT_s[:, j, b * 512:(b + 1) * 512], ps)

    def proj_v(c):
        ps = ps_av.tile([128, 512], F32, tag="av", name=f"ps_v{c}")
        for d in range(DC):
            nc.tensor.matmul(
                ps, xT_s[:, d, c * 128:(c + 1) * 128], wv_s[:, d, :],
                start=(d == 0), stop=(d == DC - 1))
        nc.vector.tensor_copy(
            vp_s[:, c, :, 0:DH],
            ps.rearrange("p (h e) -> p h e", h=H))

    # ---- warmup: junk matmuls to lift the PE HAM clock gate and a junk
    # exp to pull the ACT table load off the critical path, all during DMA.
    warm = const.tile([128, 512], BF)
    nc.vector.memset(warm, 1.0)
    wps = ps_av.tile([128, 512], F32, tag="av", name="wps")
    for i in range(18):
        nc.tensor.matmul(wps, warm[:, 0:128], warm, start=(i == 0),
                         stop=(i == 17))
    warm_out = const.tile([1, 32], BF)
    nc.scalar.activation(warm_out, wps[0:1, 0:32], EXP, bias=0.0, scale=0.0)

    # kT/qT for the first head pair's first blocks up front; everything else
    # fills PE gaps inside the attention sweeps.
    proj_kT(0, 0)
    proj_qT(0, 0)

    def normalize(p, qb, sA, sB):
        # out_allT[head rows] = staged out' * (1/denominator); runs entirely
        # from the SBUF staging copies, off the accumulation critical path.
        # Both denominator rows go into one tile at quadrant-aligned
        # partitions 0 and 32, so the expensive iterative reciprocal (8
        # cycles/element, per-lane) runs ONCE for both heads. (The custom-DVE
        # reciprocal_approx_fast returns garbage on this runtime path — HW
        # NaN despite CoreSim passing — so the exact op stays.)
        r2 = norm_sb.tile([33, 512], F32, tag="r2", name="r2")
        nc.vector.memset(r2, 1.0)
        nc.vector.tensor_copy(r2[0:1, :], sA[DH:DH + 1, :])
        nc.vector.tensor_copy(r2[32:33, :], sB[DH:DH + 1, :])
        rr = norm_sb.tile([33, 512], F32, tag="rr", name="rr")
        nc.vector.reciprocal(rr, r2)
        rrb = norm_sb.tile([33, 512], BF, tag="rrb", name="rrb")
        nc.vector.tensor_copy(rrb, rr)
        # broadcast tiles borrow a rotating av-pool slot; the accumulators
        # were already staged to SBUF, so no slot-release cycle exists
        bc = ps_av.tile([128, 512], F32, tag="av", name="bc")
        nc.tensor.matmul(bc[0:64, :], ones33[0:1, 0:64], rrb[0:1, :],
                         start=True, stop=True)
        nc.tensor.matmul(bc[64:128, :], ones33[32:33, 0:64], rrb[32:33, :],
                         start=True, stop=True)
        bc_sA = norm_sb.tile([64, 512], F32, tag="bcsA", name="bc_sA")
        bc_sB = norm_sb.tile([64, 512], F32, tag="bcsB", name="bc_sB")
        nc.vector.tensor_copy(bc_sA, bc[0:64, :])
        nc.vector.tensor_copy(bc_sB, bc[64:128, :])
        nc.vector.tensor_mul(
            out_allT[0:64, p, qb * 512:(qb + 1) * 512],
            sA[0:DH, :], bc_sA)
        nc.vector.tensor_mul(
            out_allT[64:128, p, qb * 512:(qb + 1) * 512],
            sB[0:DH, :], bc_sB)

    def outproj_t(t):
        of = out_sb.tile([128, D_OUT], F32, tag="of", name="of")
        for f in range(D_OUT // 512):
            po = ps_av.tile([128, 512], F32, tag="av", name=f"po{t}{f}")
            nc.tensor.matmul(po, ones_row, bo_bf[0:1, f * 512:(f + 1) * 512],
                             start=True, stop=False)
            for j in range(EC):
                nc.tensor.matmul(
                    po, out_allT[:, j, t * 128:(t + 1) * 128],
                    wo_s[:, j, f * 512:(f + 1) * 512],
                    start=False, stop=(j == EC - 1))
            nc.vector.tensor_copy(of[:, f * 512:(f + 1) * 512], po)
        nc.sync.dma_start(out[t * 128:(t + 1) * 128, :], of)

    # work queues drained inside the attention chunk loops
    pending_norm = []   # deferred normalize closures (emit mid-next-block)
    fill_q = []         # projection / outproj groups to slot into PE gaps

    def pop_fill():
        kind, j, b = fill_q.pop(0)
        if kind == "k":
            proj_kT(j, b)
        elif kind == "q":
            proj_qT(j, b)
        else:
            outproj_t(j)

    # attention sweep: query-block outer, head-pair inner. AV matmuls run
    # two chunks behind the score matmuls so their exp-dependency waits are
    # pre-satisfied and LDWEIGHTS stays pipelined. Accumulators are staged
    # to SBUF at block end so the next block's accumulators get PSUM slots
    # within ~1.5us; normalize runs from staging, deferred into the middle
    # of the following block.
    at_l = [None] * 4
    for qb in range(NQB):
        for p in range(EC):
            hA, hB = 2 * p, 2 * p + 1
            if qb == 0 and p == 0:
                fill_q.extend([("k", 0, b) for b in range(1, NKB)]
                              + [("k", 1, b) for b in range(NKB)] + [("q", 1, 0)])
            elif qb == 0 and p == 1:
                fill_q.extend([("q", 0, 1), ("q", 1, 1)]
                              + [("k", 2, b) for b in range(NKB)] + [("q", 2, 0)])
            elif qb == 0 and p == 2:
                fill_q.extend([("q", 2, 1)]
                              + [("k", 3, b) for b in range(NKB)]
                              + [("q", 3, 0), ("q", 3, 1)])
            elif qb == 1 and p < 2:
                fill_q.extend([("o", 2 * p + i, None) for i in range(2)])
            oA = ps_av.tile([128, 512], F32, tag="av", name=f"oA{p}{qb}")
            oB = ps_av.tile([128, 512], F32, tag="av", name=f"oB{p}{qb}")
            for c in range(KC + 2):
                cc = c - 2
                if cc >= 0:
                    nc.tensor.matmul(oA[0:DH + 1, :], vp_s[:, cc, hA, :],
                                     at_l[cc % 4][:, 0:512],
                                     start=(cc == 0), stop=(cc == KC - 1))
                    nc.tensor.matmul(oB[0:DH + 1, :], vp_s[:, cc, hB, :],
                                     at_l[cc % 4][:, 512:1024],
                                     start=(cc == 0), stop=(cc == KC - 1))
                if c < KC:
                    sT = ps_st.tile([128, 1024], F32, tag="st", name="sT")
                    nc.tensor.matmul(
                        sT[:, 0:512],
                        kT_s[0:64, p, c * 128:(c + 1) * 128],
                        qT_s[0:64, p, qb * 512:(qb + 1) * 512],
                        start=True, stop=True)
                    nc.tensor.matmul(
                        sT[:, 512:1024],
                        kT_s[64:128, p, c * 128:(c + 1) * 128],
                        qT_s[64:128, p, qb * 512:(qb + 1) * 512],
                        start=True, stop=True)
                    at = attn_sb.tile([128, 1024], BF, tag="at", name="at")
                    at_l[c % 4] = at
                    nc.scalar.activation(at, sT, EXP,
                                         bias=maskb_s[:, c:c + 1], scale=SCALE)
                    if qb == 0 and p == 0:
                        proj_v(c)
                if c == 5 and pending_norm:
                    pending_norm.pop(0)()
                if fill_q and c >= (7 if qb == 1 else 3):
                    # pop on odd c; also on even c if the queue would not
                    # drain within this block otherwise
                    odd_left = (KC + 1 - c) // 2 + 1
                    if c % 2 == 1 or len(fill_q) > odd_left:
                        pop_fill()
            sA = stage_sb.tile([DH + 1, 512], F32, tag="sA", name="sA")
            sB = stage_sb.tile([DH + 1, 512], F32, tag="sB", name="sB")
            nc.vector.tensor_copy(sA, oA[0:DH + 1, :])
            nc.vector.tensor_copy(sB, oB[0:DH + 1, :])
            pending_norm.append(
                lambda p=p, qb=qb, sA=sA, sB=sB: normalize(p, qb, sA, sB))

    while pending_norm:
        pending_norm.pop(0)()
    while fill_q:
        pop_fill()
    # ---- remaining output projection (nq tiles needing qb=1 columns)
    for t in range(NQ // 256, NQ // 128):
        outproj_t(t)


def _build():
    nc = bacc.Bacc("TRN2", target_bir_lowering=False, debug=False,
                   num_devices=NCORES)
    aps = dict(
        xT=nc.dram_tensor("xT", [DC, 128, N], BF, kind="ExternalInput").ap(),
        wqT=nc.dram_tensor("wqT", [128, DC, INNER], BF, kind="ExternalInput").ap(),
        wkT=nc.dram_tensor("wkT", [128, DC, INNER], BF, kind="ExternalInput").ap(),
        wvT=nc.dram_tensor("wvT", [128, DC, INNER], BF, kind="ExternalInput").ap(),
        woT=nc.dram_tensor("woT", [128, EC, D_OUT], BF, kind="ExternalInput").ap(),
        bo=nc.dram_tensor("bo", [1, D_OUT], F32, kind="ExternalInput").ap(),
        maskb=nc.dram_tensor("maskb", [128, KC], F32, kind="ExternalInput").ap(),
        out=nc.dram_tensor("out", [NQ, D_OUT], F32, kind="ExternalOutput").ap(),
    )
    with tile.TileContext(nc) as tc:
        with ExitStack() as ctx:
            _emit(ctx, tc, **aps)
    nc.compile()
    return nc


_prog = None


def _get_prog():
    global _prog
    if _prog is None:
        _prog = _build()
    return _prog


def _make_in_maps(x, Wq, Wk, Wv, Wo, bo, mask):
    bf = ml_dtypes.bfloat16
    f32 = np.float32

    def wlayout(w, chunks):
        # [out, in] -> partition-major [128, chunks, out]
        t = np.asarray(w).T.astype(bf).reshape(chunks, 128, w.shape[0])
        return np.ascontiguousarray(t.transpose(1, 0, 2))

    wqT = wlayout(Wq, DC)
    wkT = wlayout(Wk, DC)
    wvT = wlayout(Wv, DC)
    woT = wlayout(Wo, EC)
    bo2 = np.ascontiguousarray(bo).astype(f32).reshape(1, D_OUT)
    in_maps = []
    for c in range(NCORES):
        l, qh = c // 2, c % 2
        # key order per core: own query half first (q proj reads cols 0..NQ)
        perm = np.r_[qh * NQ:(qh + 1) * NQ, (1 - qh) * NQ:(2 - qh) * NQ]
        xTl = np.ascontiguousarray(x[l][perm].T.astype(bf).reshape(DC, 128, N))
        mb = np.where(mask[l][perm], 0.0, MASK_NEG).astype(f32)
        mb = np.ascontiguousarray(mb.reshape(KC, 128).T)
        in_maps.append(dict(xT=xTl, wqT=wqT, wkT=wkT, wvT=wvT,
                            woT=woT, bo=bo2, maskb=mb))
    return in_maps


def run(x, Wq, Wk, Wv, Wo, bo, mask, trace=False, tmpdir=None):
    nc = _get_prog()
    in_maps = _make_in_maps(x, Wq, Wk, Wv, Wo, bo, mask)
    res = run_bass_kernel_spmd(nc, in_maps, core_ids=list(range(NCORES)),
                               trace=trace, tmpdir=tmpdir)
    out = np.empty((L, N, D_OUT), np.float32)
    for c in range(NCORES):
        l, qh = c // 2, c % 2
        out[l, qh * NQ:(qh + 1) * NQ, :] = res.results[c]["out"]
    return out, res


def kernel(x, Wq, Wk, Wv, Wo, bo, mask):
    out, _ = run(np.asarray(x, np.float32), np.asarray(Wq, np.float32),
                 np.asarray(Wk, np.float32), np.asarray(Wv, np.float32),
                 np.asarray(Wo, np.float32), np.asarray(bo, np.float32),
                 np.asarray(mask))
    return out



# revision 5
# speedup vs baseline: 1.2287x; 1.2287x over previous
"""Cross-attention kernel for 8 Trainium2 NeuronCores (Bass/Tile).

Sharding: (L, head-half) tensor parallel. Core c handles batch l = c//2 and
heads [4*(c%2), 4*(c%2)+4) for ALL 2048 queries. Each core projects Q/K/V
only for its 4 heads (no duplicated projection work) and emits a PARTIAL
output y_part = attn_out @ Wo[:, head-block]^T; the host sums the two
partials of each pair and adds the bias during unshard.

Per-core dataflow (matmuls in bf16 with f32 PSUM accumulation):
  qT[e, n], kT[e, n]      transposed projections (feature dim on partitions)
  v'[nk, h, 65]           v projection + a ones column per head (softmax denom)
  sT[nk, nq] = kT.T @ qT  per head -- scores transposed, keys on partitions;
                          head pairs run row-packed on the PE
  attnT = exp(SCALE*sT + maskbias[nk])  one ACT op fuses scale+mask+exp+cast
  out'[65, nq] = v'.T @ attnT   rows 0..63: head out^T, row 64: denominator
  normalize split in two: the DVE reciprocal chain pops early in the next
    block; the PE broadcast matmuls + DVE muls pop mid-next-block, so the
    in-order PE queue never head-of-line blocks on the slow reciprocal
  y_part = out_allT.T @ WoT     final projection (no bias; host adds it)
"""

import numpy as np
import ml_dtypes
from contextlib import ExitStack

import concourse.bass as bass
import concourse.tile as tile
from concourse import bacc, mybir
from concourse.bass_utils import run_bass_kernel_spmd

L, N, D_IN = 4, 2048, 1024
H, DH = 8, 64
INNER = H * DH          # 512
D_OUT = D_IN
SCALE = DH ** -0.5      # 0.125
NCORES = 8
HH = H // 2             # 4 heads per core
HI = HH * DH            # 256 inner features per core
DC = D_IN // 128        # 8 contraction chunks for the projections
EC = HI // 128          # 2 feature chunks (= head pairs) per core
KC = N // 128           # 16 key chunks
NB = N // 512           # 4 key/query 512-blocks
NQB = NB                # 4 query blocks per core (all 2048 queries)
MASK_NEG = -50.0

BF = mybir.dt.bfloat16
F32 = mybir.dt.float32
EXP = mybir.ActivationFunctionType.Exp


def _emit(ctx, tc, xT, wqT, wkT, wvT, woT, maskb, out):
    nc = tc.nc

    const = ctx.enter_context(tc.tile_pool(name="const", bufs=1))
    big = ctx.enter_context(tc.tile_pool(name="big", bufs=1))
    attn_sb = ctx.enter_context(tc.tile_pool(name="attn_sb", bufs=4))
    norm_sb = ctx.enter_context(tc.tile_pool(name="norm_sb", bufs=2))
    stage_sb = ctx.enter_context(tc.tile_pool(name="stage_sb", bufs=4))
    out_sb = ctx.enter_context(tc.tile_pool(name="out_sb", bufs=2))
    ps_st = ctx.enter_context(tc.tile_pool(name="ps_st", bufs=2, space="PSUM"))
    ps_o = ctx.enter_context(tc.tile_pool(name="ps_o", bufs=2, space="PSUM"))
    ps_f = ctx.enter_context(tc.tile_pool(name="ps_f", bufs=2, space="PSUM"))

    # ---- inputs -> SBUF. Weights are partition-major (one fat DMA each);
    # x arrives as [b-block, d-chunk, 128, 512] so every (b, d) tile is one
    # contiguous 128KB transfer. Order matters: the presweep K projection
    # needs wk + xT(b) ASAP, the first attention block needs wv + maskb.
    wk_s = const.tile([128, DC, HI], BF)
    wq_s = const.tile([128, DC, HI], BF)
    wv_s = const.tile([128, DC, HI], BF)
    wo_s = const.tile([128, EC, D_OUT], BF)
    maskb_s = const.tile([128, KC], F32)
    xT_s = big.tile([128, DC, N], BF)
    nc.sync.dma_start(wk_s, wkT)
    for d in range(DC):
        nc.sync.dma_start(xT_s[:, d, 0:512], xT[0][d])
    nc.sync.dma_start(wv_s, wvT)
    nc.sync.dma_start(maskb_s, maskb)
    nc.sync.dma_start(wq_s, wqT)
    for b in range(1, NB):
        for d in range(DC):
            nc.sync.dma_start(xT_s[:, d, b * 512:(b + 1) * 512], xT[b][d])
    nc.sync.dma_start(wo_s, woT)

    ones33 = const.tile([33, 128], BF)
    nc.vector.memset(ones33, 1.0)

    kT_s = big.tile([128, EC, N], BF)
    qT_s = big.tile([128, EC, N], BF)
    vp_s = big.tile([128, KC, HH, DH + 1], BF)
    nc.vector.memset(vp_s[:, :, :, DH], 1.0)
    out_allT = big.tile([128, EC, N], BF)
    # persistent reciprocal input; rows other than 0 and 32 stay 1.0 forever
    r2 = big.tile([33, 512], F32)
    nc.vector.memset(r2, 1.0)

    def proj_kT(j, b):
        ps = ps_f.tile([128, 512], F32, tag="f", name=f"ps_k{j}{b}")
        for d in range(DC):
            nc.tensor.matmul(
                ps, wk_s[:, d, j * 128:(j + 1) * 128],
                xT_s[:, d, b * 512:(b + 1) * 512],
                start=(d == 0), stop=(d == DC - 1))
        nc.vector.tensor_copy(kT_s[:, j, b * 512:(b + 1) * 512], ps)

    def proj_qT(j, b):
        ps = ps_f.tile([128, 512], F32, tag="f", name=f"ps_q{j}{b}")
        for d in range(DC):
            nc.tensor.matmul(
                ps, wq_s[:, d, j * 128:(j + 1) * 128],
                xT_s[:, d, b * 512:(b + 1) * 512],
                start=(d == 0), stop=(d == DC - 1))
        nc.vector.tensor_copy(qT_s[:, j, b * 512:(b + 1) * 512], ps)

    def proj_v(c):
        ps = ps_f.tile([128, 512], F32, tag="f", name=f"ps_v{c}")
        for d in range(DC):
            nc.tensor.matmul(
                ps[:, 0:HI], xT_s[:, d, c * 128:(c + 1) * 128], wv_s[:, d, :],
                start=(d == 0), stop=(d == DC - 1))
        nc.vector.tensor_copy(
            vp_s[:, c, :, 0:DH],
            ps[:, 0:HI].rearrange("p (h e) -> p h e", h=HH))

    # ---- warmup: junk matmuls lift the PE p-state clock gate during the
    # input DMA; a junk exp pulls the ACT table load off the critical path.
    warm = const.tile([128, 512], BF)
    nc.vector.memset(warm, 1.0)
    wps = ps_f.tile([128, 512], F32, tag="f", name="wps")
    for i in range(24):
        nc.tensor.matmul(wps, warm[:, 0:128], warm, start=(i == 0),
                         stop=(i == 23))
    warm_out = const.tile([1, 32], BF)
    nc.scalar.activation(warm_out, wps[0:1, 0:32], EXP, bias=0.0, scale=0.0)

    # presweep projections (DMA-gated): K for head pair 0, first q blocks
    for b in range(NB):
        proj_kT(0, b)
    proj_qT(0, 0)
    proj_qT(1, 0)

    def norm_recip(sA, sB):
        # 1/denominator for both heads; rows 0 and 32 of the persistent r2
        # (quadrant-aligned so one iterative reciprocal serves both heads,
        # and so the two broadcast matmuls get distinct PE row quadrants)
        nc.vector.tensor_copy(r2[0:1, :], sA[DH:DH + 1, :])
        nc.vector.tensor_copy(r2[32:33, :], sB[DH:DH + 1, :])
        rr = norm_sb.tile([33, 512], F32, tag="rr", name="rr")
        nc.vector.reciprocal(rr, r2)
        rrb = norm_sb.tile([33, 512], BF, tag="rrb", name="rrb")
        nc.vector.tensor_copy(rrb, rr)
        return rrb

    def norm_apply(p, qb, sA, sB, rrb):
        # out_allT[head rows] = staged out' * (1/denominator); the PE part
        # (broadcast matmuls) only lands here, after rrb is long done.
        bc = ps_f.tile([128, 512], F32, tag="f", name="bc")
        nc.tensor.matmul(bc[0:64, :], ones33[0:1, 0:64], rrb[0:1, :],
                         start=True, stop=True)
        nc.tensor.matmul(bc[64:128, :], ones33[32:33, 0:64], rrb[32:33, :],
                         start=True, stop=True)
        bc_sA = norm_sb.tile([64, 512], F32, tag="bcsA", name="bc_sA")
        bc_sB = norm_sb.tile([64, 512], F32, tag="bcsB", name="bc_sB")
        nc.vector.tensor_copy(bc_sA, bc[0:64, :])
        nc.vector.tensor_copy(bc_sB, bc[64:128, :])
        nc.vector.tensor_mul(
            out_allT[0:64, p, qb * 512:(qb + 1) * 512], sA[0:DH, :], bc_sA)
        nc.vector.tensor_mul(
            out_allT[64:128, p, qb * 512:(qb + 1) * 512], sB[0:DH, :], bc_sB)

    def outproj_t(t):
        of = out_sb.tile([128, D_OUT], BF, tag="of", name="of")
        for f in range(D_OUT // 512):
            po = ps_f.tile([128, 512], F32, tag="f", name=f"po{t}{f}")
            for j in range(EC):
                nc.tensor.matmul(
                    po, out_allT[:, j, t * 128:(t + 1) * 128],
                    wo_s[:, j, f * 512:(f + 1) * 512],
                    start=(j == 0), stop=(j == EC - 1))
            nc.vector.tensor_copy(of[:, f * 512:(f + 1) * 512], po)
        nc.sync.dma_start(out[t * 128:(t + 1) * 128, :], of)

    def K(j, b):
        return lambda: proj_kT(j, b)

    def Q(j, b):
        return lambda: proj_qT(j, b)

    def O(t):
        return lambda: outproj_t(t)

    # per-block fill plan: "early" pops at c=3,5,7..., "late" at c=13,15.
    # K(1,*) must complete inside B0/B1 before their score chunks; O(t) for
    # query block qb is safe only two blocks after (qb,p1)'s norm_apply.
    FILLS = {
        0: ([K(1, 0), K(1, 1)], []),
        1: ([K(1, 2), K(1, 3)], [Q(0, 1), Q(1, 1)]),
        2: ([Q(0, 2), Q(1, 2)], [O(0), O(1)]),
        3: ([O(2), O(3)], [Q(0, 3), Q(1, 3)]),
        4: ([], [O(4), O(5)]),
        5: ([O(6), O(7)], []),
        6: ([], [O(8), O(9)]),
        7: ([O(10), O(11)], []),
    }

    # attention sweep: query-block outer, head-pair inner. AV matmuls run
    # two chunks behind the score matmuls so their exp-dependency waits are
    # pre-satisfied. Accumulators are staged to SBUF at block end so the
    # next block's accumulators get PSUM slots quickly; the normalize runs
    # from staging, split across the following block.
    pending = []        # (p, qb, sA, sB, rrb) through the two norm stages
    at_l = [None] * 4
    for bi in range(NQB * EC):
        qb, p = bi // EC, bi % EC
        hA, hB = 2 * p, 2 * p + 1
        early, late = FILLS[bi]
        early = list(early)
        late = list(late)
        oA = ps_o.tile([DH + 1, 512], F32, tag="o", name=f"oA{bi}")
        oB = ps_o.tile([DH + 1, 512], F32, tag="o", name=f"oB{bi}")
        for c in range(KC + 2):
            cc = c - 2
            if cc >= 0:
                nc.tensor.matmul(oA, vp_s[:, cc, hA, :],
                                 at_l[cc % 4][:, 0:512],
                                 start=(cc == 0), stop=(cc == KC - 1))
                nc.tensor.matmul(oB, vp_s[:, cc, hB, :],
                                 at_l[cc % 4][:, 512:1024],
                                 start=(cc == 0), stop=(cc == KC - 1))
            if c < KC:
                sT = ps_st.tile([128, 1024], F32, tag="st", name="sT")
                nc.tensor.matmul(
                    sT[:, 0:512],
                    kT_s[0:64, p, c * 128:(c + 1) * 128],
                    qT_s[0:64, p, qb * 512:(qb + 1) * 512],
                    start=True, stop=True)
                nc.tensor.matmul(
                    sT[:, 512:1024],
                    kT_s[64:128, p, c * 128:(c + 1) * 128],
                    qT_s[64:128, p, qb * 512:(qb + 1) * 512],
                    start=True, stop=True)
                at = attn_sb.tile([128, 1024], BF, tag="at", name="at")
                at_l[c % 4] = at
                nc.scalar.activation(at, sT, EXP,
                                     bias=maskb_s[:, c:c + 1], scale=SCALE)
                if bi == 0:
                    proj_v(c)
            if c == 1 and pending:
                pending[0] = pending[0][:4] + (
                    norm_recip(pending[0][2], pending[0][3]),)
            if c == 9 and pending:
                pp, pqb, sA, sB, rrb = pending.pop(0)
                norm_apply(pp, pqb, sA, sB, rrb)
            if c >= 3 and c % 2 == 1 and early:
                early.pop(0)()
            if c >= 13 and c % 2 == 1 and late:
                late.pop(0)()
        sA = stage_sb.tile([DH + 1, 512], F32, tag="sA", name="sA")
        sB = stage_sb.tile([DH + 1, 512], F32, tag="sB", name="sB")
        nc.vector.tensor_copy(sA, oA)
        nc.vector.tensor_copy(sB, oB)
        pending.append((p, qb, sA, sB, None))

    # ---- tail: last block's normalize, then the final query tiles
    while pending:
        pp, pqb, sA, sB, _ = pending.pop(0)
        rrb = norm_recip(sA, sB)
        norm_apply(pp, pqb, sA, sB, rrb)
    for t in range(12, 16):
        outproj_t(t)


def _build():
    nc = bacc.Bacc("TRN2", target_bir_lowering=False, debug=False,
                   num_devices=NCORES)
    aps = dict(
        xT=nc.dram_tensor("xT", [NB, DC, 128, 512], BF,
                          kind="ExternalInput").ap(),
        wqT=nc.dram_tensor("wqT", [128, DC, HI], BF, kind="ExternalInput").ap(),
        wkT=nc.dram_tensor("wkT", [128, DC, HI], BF, kind="ExternalInput").ap(),
        wvT=nc.dram_tensor("wvT", [128, DC, HI], BF, kind="ExternalInput").ap(),
        woT=nc.dram_tensor("woT", [128, EC, D_OUT], BF,
                           kind="ExternalInput").ap(),
        maskb=nc.dram_tensor("maskb", [128, KC], F32, kind="ExternalInput").ap(),
        out=nc.dram_tensor("out", [N, D_OUT], BF, kind="ExternalOutput").ap(),
    )
    with tile.TileContext(nc) as tc:
        with ExitStack() as ctx:
            _emit(ctx, tc, **aps)
    nc.compile()
    return nc


_prog = None


def _get_prog():
    global _prog
    if _prog is None:
        _prog = _build()
    return _prog


def _make_in_maps(x, Wq, Wk, Wv, Wo, bo, mask):
    bf = ml_dtypes.bfloat16
    f32 = np.float32

    def wlayout(w):
        # [256 out, in] -> partition-major [128, in//128, 256]
        t = np.asarray(w).T.astype(bf).reshape(-1, 128, w.shape[0])
        return np.ascontiguousarray(t.transpose(1, 0, 2))

    in_maps = []
    for c in range(NCORES):
        l, hh = c // 2, c % 2
        sl = slice(hh * HI, (hh + 1) * HI)
        xTl = np.ascontiguousarray(
            x[l].T.astype(bf).reshape(DC, 128, NB, 512).transpose(2, 0, 1, 3))
        mb = np.where(mask[l], 0.0, MASK_NEG).astype(f32)
        mb = np.ascontiguousarray(mb.reshape(KC, 128).T)
        woT = np.ascontiguousarray(
            Wo[:, sl].T.astype(bf).reshape(EC, 128, D_OUT).transpose(1, 0, 2))
        in_maps.append(dict(xT=xTl, wqT=wlayout(Wq[sl]), wkT=wlayout(Wk[sl]),
                            wvT=wlayout(Wv[sl]), woT=woT, maskb=mb))
    return in_maps


def run(x, Wq, Wk, Wv, Wo, bo, mask, trace=False, tmpdir=None):
    nc = _get_prog()
    in_maps = _make_in_maps(x, Wq, Wk, Wv, Wo, bo, mask)
    res = run_bass_kernel_spmd(nc, in_maps, core_ids=list(range(NCORES)),
                               trace=trace, tmpdir=tmpdir)
    out = np.empty((L, N, D_OUT), np.float32)
    bo_f = np.asarray(bo, np.float32)
    for l in range(L):
        out[l] = (res.results[2 * l]["out"].astype(np.float32)
                  + res.results[2 * l + 1]["out"].astype(np.float32) + bo_f)
    return out, res


def kernel(x, Wq, Wk, Wv, Wo, bo, mask):
    out, _ = run(np.asarray(x, np.float32), np.asarray(Wq, np.float32),
                 np.asarray(Wk, np.float32), np.asarray(Wv, np.float32),
                 np.asarray(Wo, np.float32), np.asarray(bo, np.float32),
                 np.asarray(mask))
    return out


# revision 14
# speedup vs baseline: 1.2390x; 1.0085x over previous
"""Cross-attention kernel for 8 Trainium2 NeuronCores (Bass/Tile).

Sharding: (L, head-half) tensor parallel. Core c handles batch l = c//2 and
heads [4*(c%2), 4*(c%2)+4) for ALL 2048 queries. Each core projects Q/K/V
only for its 4 heads (no duplicated projection work) and emits a PARTIAL
output y_part = attn_out @ Wo[:, head-block]^T; the host sums the two
partials of each pair and adds the bias during unshard.

Per-core dataflow (matmuls in bf16 with f32 PSUM accumulation):
  qT[e, n], kT[e, n]      transposed projections (feature dim on partitions)
  v'[nk, h, 65]           v projection + a ones column per head (softmax denom)
  sT[nk, nq] = kT.T @ qT  per head -- scores transposed, keys on partitions;
                          head pairs run row-packed on the PE
  attnT = exp(SCALE*sT + maskbias[nk])  one ACT op fuses scale+mask+exp+cast
  out'[65, nq] = v'.T @ attnT   rows 0..63: head out^T, row 64: denominator
  normalize split in two: the DVE reciprocal chain pops early in the next
    block; the PE broadcast matmuls + DVE muls pop mid-next-block, so the
    in-order PE queue never head-of-line blocks on the slow reciprocal
  y_part = out_allT.T @ WoT     final projection (no bias; host adds it)
"""

import numpy as np
import ml_dtypes
from contextlib import ExitStack

import concourse.bass as bass
import concourse.tile as tile
from concourse import bacc, mybir
from concourse.bass_utils import run_bass_kernel_spmd

L, N, D_IN = 4, 2048, 1024
H, DH = 8, 64
INNER = H * DH          # 512
D_OUT = D_IN
SCALE = DH ** -0.5      # 0.125
NCORES = 8
HH = H // 2             # 4 heads per core
HI = HH * DH            # 256 inner features per core
DC = D_IN // 128        # 8 contraction chunks for the projections
EC = HI // 128          # 2 feature chunks (= head pairs) per core
KC = N // 128           # 16 key chunks
NB = N // 512           # 4 key/query 512-blocks
NQB = NB                # 4 query blocks per core (all 2048 queries)
MASK_NEG = -50.0

BF = mybir.dt.bfloat16
F32 = mybir.dt.float32
EXP = mybir.ActivationFunctionType.Exp


def _emit(ctx, tc, xT, wqT, wkT, wvT, woT, maskb, out):
    nc = tc.nc

    const = ctx.enter_context(tc.tile_pool(name="const", bufs=1))
    big = ctx.enter_context(tc.tile_pool(name="big", bufs=1))
    attn_sb = ctx.enter_context(tc.tile_pool(name="attn_sb", bufs=4))
    norm_sb = ctx.enter_context(tc.tile_pool(name="norm_sb", bufs=2))
    stage_sb = ctx.enter_context(tc.tile_pool(name="stage_sb", bufs=4))
    out_sb = ctx.enter_context(tc.tile_pool(name="out_sb", bufs=2))
    ps_st = ctx.enter_context(tc.tile_pool(name="ps_st", bufs=2, space="PSUM"))
    ps_o = ctx.enter_context(tc.tile_pool(name="ps_o", bufs=2, space="PSUM"))
    ps_f = ctx.enter_context(tc.tile_pool(name="ps_f", bufs=2, space="PSUM"))

    # ---- inputs -> SBUF. Weights are partition-major (one fat DMA each);
    # x arrives as [b-block, d-chunk, 128, 512] so every (b, d) tile is one
    # contiguous 128KB transfer. Order matters: the presweep K projection
    # needs wk + xT(b) ASAP, the first attention block needs wv + maskb.
    wk_s = const.tile([128, DC, HI], BF)
    wq_s = const.tile([128, DC, HI], BF)
    wv_s = const.tile([128, DC, HI], BF)
    wo_s = const.tile([128, EC, D_OUT], BF)
    maskb_s = const.tile([128, KC], F32)
    xT_s = big.tile([128, DC, N], BF)
    nc.sync.dma_start(wk_s, wkT)
    nc.sync.dma_start(xT_s[:, :, 0:512], xT[0].rearrange("d p c -> p d c"))
    nc.sync.dma_start(wv_s, wvT)
    nc.sync.dma_start(maskb_s, maskb)
    nc.sync.dma_start(wq_s, wqT)
    for b in range(1, NB):
        nc.sync.dma_start(xT_s[:, :, b * 512:(b + 1) * 512],
                          xT[b].rearrange("d p c -> p d c"))
    nc.sync.dma_start(wo_s, woT)

    ones33 = const.tile([33, 128], BF)
    nc.vector.memset(ones33, 1.0)

    kT_s = big.tile([128, EC, N], BF)
    qT_s = big.tile([128, EC, N], BF)
    vp_s = big.tile([128, KC, HH, DH + 1], BF)
    nc.vector.memset(vp_s[:, :, :, DH], 1.0)
    out_allT = big.tile([128, EC, N], BF)
    # persistent reciprocal input; rows other than 0 and 32 stay 1.0 forever
    r2 = big.tile([33, 512], F32)
    nc.vector.memset(r2, 1.0)

    def proj_kT(j, b):
        ps = ps_f.tile([128, 512], F32, tag="f", name=f"ps_k{j}{b}")
        for d in range(DC):
            nc.tensor.matmul(
                ps, wk_s[:, d, j * 128:(j + 1) * 128],
                xT_s[:, d, b * 512:(b + 1) * 512],
                start=(d == 0), stop=(d == DC - 1))
        nc.vector.tensor_copy(kT_s[:, j, b * 512:(b + 1) * 512], ps)

    def proj_qT(j, b):
        ps = ps_f.tile([128, 512], F32, tag="f", name=f"ps_q{j}{b}")
        for d in range(DC):
            nc.tensor.matmul(
                ps, wq_s[:, d, j * 128:(j + 1) * 128],
                xT_s[:, d, b * 512:(b + 1) * 512],
                start=(d == 0), stop=(d == DC - 1))
        nc.vector.tensor_copy(qT_s[:, j, b * 512:(b + 1) * 512], ps)

    def proj_v(c):
        ps = ps_f.tile([128, 512], F32, tag="f", name=f"ps_v{c}")
        for d in range(DC):
            nc.tensor.matmul(
                ps[:, 0:HI], xT_s[:, d, c * 128:(c + 1) * 128], wv_s[:, d, :],
                start=(d == 0), stop=(d == DC - 1))
        nc.vector.tensor_copy(
            vp_s[:, c, :, 0:DH],
            ps[:, 0:HI].rearrange("p (h e) -> p h e", h=HH))

    # ---- warmup: junk matmuls lift the PE p-state clock gate during the
    # input DMA; a junk exp pulls the ACT table load off the critical path.
    warm = const.tile([128, 512], BF)
    nc.vector.memset(warm, 1.0)
    wps = ps_f.tile([128, 512], F32, tag="f", name="wps")
    for i in range(14):
        nc.tensor.matmul(wps, warm[:, 0:128], warm, start=(i == 0),
                         stop=(i == 13))
    warm_out = const.tile([1, 32], BF)
    nc.scalar.activation(warm_out, wps[0:1, 0:32], EXP, bias=0.0, scale=0.0)

    # presweep projections (DMA-gated): K for head pair 0, first q blocks
    for b in range(NB):
        proj_kT(0, b)
    proj_qT(0, 0)
    proj_qT(1, 0)

    def norm_recip():
        # 1/denominator for both heads; the denominator rows were copied
        # into r2 rows 0 and 32 straight from PSUM at block end (quadrant-
        # aligned so one iterative reciprocal serves both heads, and so the
        # two broadcast matmuls get distinct PE row quadrants)
        rr = norm_sb.tile([33, 512], F32, tag="rr", name="rr")
        nc.vector.reciprocal(rr, r2)
        rrb = norm_sb.tile([33, 512], BF, tag="rrb", name="rrb")
        nc.vector.tensor_copy(rrb, rr)
        return rrb

    def norm_apply(p, qb, sA, sB, rrb):
        # out_allT[head rows] = staged out' * (1/denominator); the PE part
        # (broadcast matmuls) only lands here, after rrb is long done.
        bc = ps_f.tile([128, 512], F32, tag="f", name="bc")
        nc.tensor.matmul(bc[0:64, :], ones33[0:1, 0:64], rrb[0:1, :],
                         start=True, stop=True)
        nc.tensor.matmul(bc[64:128, :], ones33[32:33, 0:64], rrb[32:33, :],
                         start=True, stop=True)
        bc_sA = norm_sb.tile([64, 512], F32, tag="bcsA", name="bc_sA")
        bc_sB = norm_sb.tile([64, 512], F32, tag="bcsB", name="bc_sB")
        nc.vector.tensor_copy(bc_sA, bc[0:64, :])
        nc.vector.tensor_copy(bc_sB, bc[64:128, :])
        nc.vector.tensor_mul(
            out_allT[0:64, p, qb * 512:(qb + 1) * 512], sA, bc_sA)
        nc.vector.tensor_mul(
            out_allT[64:128, p, qb * 512:(qb + 1) * 512], sB, bc_sB)

    def outproj_t(t, tail=False):
        of = out_sb.tile([128, D_OUT], BF, tag="of", name="of")
        for f in range(D_OUT // 512):
            po = ps_f.tile([128, 512], F32, tag="f", name=f"po{t}{f}")
            for j in range(EC):
                nc.tensor.matmul(
                    po, out_allT[:, j, t * 128:(t + 1) * 128],
                    wo_s[:, j, f * 512:(f + 1) * 512],
                    start=(j == 0), stop=(j == EC - 1))
            # in the tail the idle Scalar engine takes half the casts so the
            # DVE is not the serial bottleneck of the last four tiles
            if tail and f == 1:
                nc.scalar.copy(of[:, f * 512:(f + 1) * 512], po)
            else:
                nc.vector.tensor_copy(of[:, f * 512:(f + 1) * 512], po)
        nc.sync.dma_start(out[t * 128:(t + 1) * 128, :], of)

    def K(j, b):
        return lambda: proj_kT(j, b)

    def Q(j, b):
        return lambda: proj_qT(j, b)

    def O(t):
        return lambda: outproj_t(t)

    # per-block fill plan: "early" pops at c=3,5,7..., "late" at c=13,15.
    # K(1,*) must complete inside B0/B1 before their score chunks; O(t) for
    # query block qb is safe only two blocks after (qb,p1)'s norm_apply.
    FILLS = {
        0: ([K(1, 0), K(1, 1)], []),
        1: ([K(1, 2), K(1, 3)], [Q(0, 1), Q(1, 1)]),
        2: ([Q(0, 2), Q(1, 2)], [O(0), O(1)]),
        3: ([O(2), O(3)], [Q(0, 3), Q(1, 3)]),
        4: ([], [O(4), O(5)]),
        5: ([O(6), O(7)], []),
        6: ([], [O(8), O(9)]),
        7: ([O(10), O(11)], []),
    }

    # attention sweep: query-block outer, head-pair inner. AV matmuls run
    # two chunks behind the score matmuls so their exp-dependency waits are
    # pre-satisfied. Accumulators are staged to SBUF at block end so the
    # next block's accumulators get PSUM slots quickly; the normalize runs
    # from staging, split across the following block.
    pending = []        # (p, qb, sA, sB, rrb) through the two norm stages
    at_l = [None] * 4
    for bi in range(NQB * EC):
        qb, p = bi // EC, bi % EC
        hA, hB = 2 * p, 2 * p + 1
        early, late = FILLS[bi]
        early = list(early)
        late = list(late)
        oA = ps_o.tile([DH + 1, 512], F32, tag="o", name=f"oA{bi}")
        oB = ps_o.tile([DH + 1, 512], F32, tag="o", name=f"oB{bi}")
        for c in range(KC + 2):
            cc = c - 2
            if cc >= 0:
                nc.tensor.matmul(oA, vp_s[:, cc, hA, :],
                                 at_l[cc % 4][:, 0:512],
                                 start=(cc == 0), stop=(cc == KC - 1))
                nc.tensor.matmul(oB, vp_s[:, cc, hB, :],
                                 at_l[cc % 4][:, 512:1024],
                                 start=(cc == 0), stop=(cc == KC - 1))
            if c < KC:
                sT = ps_st.tile([128, 1024], F32, tag="st", name="sT")
                nc.tensor.matmul(
                    sT[:, 0:512],
                    kT_s[0:64, p, c * 128:(c + 1) * 128],
                    qT_s[0:64, p, qb * 512:(qb + 1) * 512],
                    start=True, stop=True)
                nc.tensor.matmul(
                    sT[:, 512:1024],
                    kT_s[64:128, p, c * 128:(c + 1) * 128],
                    qT_s[64:128, p, qb * 512:(qb + 1) * 512],
                    start=True, stop=True)
                at = attn_sb.tile([128, 1024], BF, tag="at", name="at")
                at_l[c % 4] = at
                nc.scalar.activation(at, sT, EXP,
                                     bias=maskb_s[:, c:c + 1], scale=SCALE)
                if bi == 0:
                    proj_v(c)
            if c == 1 and pending:
                pending[0] = pending[0][:4] + (norm_recip(),)
            if c == 13 and pending:
                pp, pqb, sA, sB, rrb = pending.pop(0)
                norm_apply(pp, pqb, sA, sB, rrb)
            if c >= 3 and c % 2 == 1 and early:
                early.pop(0)()
            if c >= 15 and c % 2 == 1 and late:
                late.pop(0)()
        # denominator rows straight from PSUM so the reciprocal chain is
        # not serialized behind the (bigger) stage copies
        nc.vector.tensor_copy(r2[0:1, :], oA[DH:DH + 1, :])
        nc.vector.tensor_copy(r2[32:33, :], oB[DH:DH + 1, :])
        sA = stage_sb.tile([DH, 512], F32, tag="sA", name="sA")
        sB = stage_sb.tile([DH, 512], F32, tag="sB", name="sB")
        nc.vector.tensor_copy(sA, oA[0:DH, :])
        nc.vector.tensor_copy(sB, oB[0:DH, :])
        pending.append((p, qb, sA, sB, None))

    # ---- tail: last block's normalize, then the final query tiles
    while pending:
        pp, pqb, sA, sB, rrb = pending.pop(0)
        if rrb is None:
            rrb = norm_recip()
        norm_apply(pp, pqb, sA, sB, rrb)
    for t in range(12, 16):
        outproj_t(t, tail=True)


def _build():
    nc = bacc.Bacc("TRN2", target_bir_lowering=False, debug=False,
                   num_devices=NCORES)
    aps = dict(
        xT=nc.dram_tensor("xT", [NB, DC, 128, 512], BF,
                          kind="ExternalInput").ap(),
        wqT=nc.dram_tensor("wqT", [128, DC, HI], BF, kind="ExternalInput").ap(),
        wkT=nc.dram_tensor("wkT", [128, DC, HI], BF, kind="ExternalInput").ap(),
        wvT=nc.dram_tensor("wvT", [128, DC, HI], BF, kind="ExternalInput").ap(),
        woT=nc.dram_tensor("woT", [128, EC, D_OUT], BF,
                           kind="ExternalInput").ap(),
        maskb=nc.dram_tensor("maskb", [128, KC], F32, kind="ExternalInput").ap(),
        out=nc.dram_tensor("out", [N, D_OUT], BF, kind="ExternalOutput").ap(),
    )
    with tile.TileContext(nc) as tc:
        with ExitStack() as ctx:
            _emit(ctx, tc, **aps)
    nc.compile()
    return nc


_prog = None


def _get_prog():
    global _prog
    if _prog is None:
        _prog = _build()
    return _prog


def _make_in_maps(x, Wq, Wk, Wv, Wo, bo, mask):
    bf = ml_dtypes.bfloat16
    f32 = np.float32

    def wlayout(w):
        # [256 out, in] -> partition-major [128, in//128, 256]
        t = np.asarray(w).T.astype(bf).reshape(-1, 128, w.shape[0])
        return np.ascontiguousarray(t.transpose(1, 0, 2))

    in_maps = []
    for c in range(NCORES):
        l, hh = c // 2, c % 2
        sl = slice(hh * HI, (hh + 1) * HI)
        xTl = np.ascontiguousarray(
            x[l].T.astype(bf).reshape(DC, 128, NB, 512).transpose(2, 0, 1, 3))
        mb = np.where(mask[l], 0.0, MASK_NEG).astype(f32)
        mb = np.ascontiguousarray(mb.reshape(KC, 128).T)
        woT = np.ascontiguousarray(
            Wo[:, sl].T.astype(bf).reshape(EC, 128, D_OUT).transpose(1, 0, 2))
        in_maps.append(dict(xT=xTl, wqT=wlayout(Wq[sl]), wkT=wlayout(Wk[sl]),
                            wvT=wlayout(Wv[sl]), woT=woT, maskb=mb))
    return in_maps


def run(x, Wq, Wk, Wv, Wo, bo, mask, trace=False, tmpdir=None):
    nc = _get_prog()
    in_maps = _make_in_maps(x, Wq, Wk, Wv, Wo, bo, mask)
    res = run_bass_kernel_spmd(nc, in_maps, core_ids=list(range(NCORES)),
                               trace=trace, tmpdir=tmpdir)
    out = np.empty((L, N, D_OUT), np.float32)
    bo_f = np.asarray(bo, np.float32)
    for l in range(L):
        out[l] = (res.results[2 * l]["out"].astype(np.float32)
                  + res.results[2 * l + 1]["out"].astype(np.float32) + bo_f)
    return out, res


def kernel(x, Wq, Wk, Wv, Wo, bo, mask):
    out, _ = run(np.asarray(x, np.float32), np.asarray(Wq, np.float32),
                 np.asarray(Wk, np.float32), np.asarray(Wv, np.float32),
                 np.asarray(Wo, np.float32), np.asarray(bo, np.float32),
                 np.asarray(mask))
    return out


# revision 25
# speedup vs baseline: 1.2486x; 1.0077x over previous
"""Cross-attention kernel for 8 Trainium2 NeuronCores (Bass/Tile).

Sharding: (L, head-half) tensor parallel. Core c handles batch l = c//2 and
heads [4*(c%2), 4*(c%2)+4) for ALL 2048 queries. Each core projects Q/K/V
only for its 4 heads (no duplicated projection work) and emits a PARTIAL
output y_part = attn_out @ Wo[:, head-block]^T; the host sums the two
partials of each pair and adds the bias during unshard.

Per-core dataflow (matmuls in bf16 with f32 PSUM accumulation):
  qT[e, n], kT[e, n]      transposed projections (feature dim on partitions)
  v'[nk, h, 65]           v projection + a ones column per head (softmax denom)
  sT[nk, nq] = kT.T @ qT  per head -- scores transposed, keys on partitions;
                          head pairs run row-packed on the PE
  attnT = exp(SCALE*sT + maskbias[nk])  one ACT op fuses scale+mask+exp+cast
  out'[65, nq] = v'.T @ attnT   rows 0..63: head out^T, row 64: denominator
  normalize split in two: the DVE reciprocal chain pops early in the next
    block; the PE broadcast matmuls + DVE muls pop mid-next-block, so the
    in-order PE queue never head-of-line blocks on the slow reciprocal
  y_part = out_allT.T @ WoT     final projection (no bias; host adds it)
"""

import numpy as np
import ml_dtypes
from contextlib import ExitStack

import concourse.bass as bass
import concourse.tile as tile
from concourse import bacc, mybir
from concourse.bass_utils import run_bass_kernel_spmd

L, N, D_IN = 4, 2048, 1024
H, DH = 8, 64
INNER = H * DH          # 512
D_OUT = D_IN
SCALE = DH ** -0.5      # 0.125
NCORES = 8
HH = H // 2             # 4 heads per core
HI = HH * DH            # 256 inner features per core
DC = D_IN // 128        # 8 contraction chunks for the projections
EC = HI // 128          # 2 feature chunks (= head pairs) per core
KC = N // 128           # 16 key chunks
NB = N // 512           # 4 key/query 512-blocks
NQB = NB                # 4 query blocks per core (all 2048 queries)
MASK_NEG = -50.0

BF = mybir.dt.bfloat16
F32 = mybir.dt.float32
EXP = mybir.ActivationFunctionType.Exp


def _emit(ctx, tc, xT, wqT, wkT, wvT, woT, maskb, out):
    nc = tc.nc

    const = ctx.enter_context(tc.tile_pool(name="const", bufs=1))
    big = ctx.enter_context(tc.tile_pool(name="big", bufs=1))
    attn_sb = ctx.enter_context(tc.tile_pool(name="attn_sb", bufs=4))
    norm_sb = ctx.enter_context(tc.tile_pool(name="norm_sb", bufs=2))
    stage_sb = ctx.enter_context(tc.tile_pool(name="stage_sb", bufs=4))
    out_sb = ctx.enter_context(tc.tile_pool(name="out_sb", bufs=2))
    ps_st = ctx.enter_context(tc.tile_pool(name="ps_st", bufs=2, space="PSUM"))
    ps_o = ctx.enter_context(tc.tile_pool(name="ps_o", bufs=2, space="PSUM"))
    ps_f = ctx.enter_context(tc.tile_pool(name="ps_f", bufs=2, space="PSUM"))

    # ---- inputs -> SBUF. Weights are partition-major (one fat DMA each);
    # x arrives as [b-block, d-chunk, 128, 512] so every (b, d) tile is one
    # contiguous 128KB transfer. Order matters: the presweep K projection
    # needs wk + xT(b) ASAP, the first attention block needs wv + maskb.
    wk_s = const.tile([128, DC, HI], BF)
    wq_s = const.tile([128, DC, HI], BF)
    wv_s = const.tile([128, DC, HI], BF)
    wo_s = const.tile([128, EC, D_OUT], BF)
    maskb_s = const.tile([128, KC], F32)
    xT_s = big.tile([128, DC, N], BF)
    nc.sync.dma_start(wk_s, wkT)
    nc.sync.dma_start(xT_s[:, :, 0:512], xT[0].rearrange("d p c -> p d c"))
    nc.sync.dma_start(wv_s, wvT)
    nc.sync.dma_start(maskb_s, maskb)
    nc.sync.dma_start(wq_s, wqT)
    for b in range(1, NB):
        nc.sync.dma_start(xT_s[:, :, b * 512:(b + 1) * 512],
                          xT[b].rearrange("d p c -> p d c"))
    nc.sync.dma_start(wo_s, woT)

    ones33 = const.tile([33, 128], BF)
    nc.vector.memset(ones33, 1.0)

    kT_s = big.tile([128, EC, N], BF)
    qT_s = big.tile([128, EC, N], BF)
    vp_s = big.tile([128, KC, HH, DH + 1], BF)
    nc.vector.memset(vp_s[:, :, :, DH], 1.0)
    out_allT = big.tile([128, EC, N], BF)
    # persistent reciprocal input; rows other than 0 and 32 stay 1.0 forever
    r2 = big.tile([33, 512], F32)
    nc.vector.memset(r2, 1.0)
    # tail fast-reciprocal scratch: 32x32 stream-transpose spreads the two
    # denominator rows across partitions so the iterative reciprocal runs on
    # 32 elements per lane instead of 512 (rows/cols never touched stay 1.0)
    td = big.tile([64, 512], F32)
    nc.vector.memset(td, 1.0)
    tR = big.tile([64, 512], F32)
    nc.vector.memset(tR, 1.0)

    def proj_kT(j, b):
        ps = ps_f.tile([128, 512], F32, tag="f", name=f"ps_k{j}{b}")
        for d in range(DC):
            nc.tensor.matmul(
                ps, wk_s[:, d, j * 128:(j + 1) * 128],
                xT_s[:, d, b * 512:(b + 1) * 512],
                start=(d == 0), stop=(d == DC - 1))
        nc.vector.tensor_copy(kT_s[:, j, b * 512:(b + 1) * 512], ps)

    def proj_qT(j, b):
        ps = ps_f.tile([128, 512], F32, tag="f", name=f"ps_q{j}{b}")
        for d in range(DC):
            nc.tensor.matmul(
                ps, wq_s[:, d, j * 128:(j + 1) * 128],
                xT_s[:, d, b * 512:(b + 1) * 512],
                start=(d == 0), stop=(d == DC - 1))
        nc.vector.tensor_copy(qT_s[:, j, b * 512:(b + 1) * 512], ps)

    def proj_v(c):
        ps = ps_f.tile([128, 512], F32, tag="f", name=f"ps_v{c}")
        for d in range(DC):
            nc.tensor.matmul(
                ps[:, 0:HI], xT_s[:, d, c * 128:(c + 1) * 128], wv_s[:, d, :],
                start=(d == 0), stop=(d == DC - 1))
        nc.vector.tensor_copy(
            vp_s[:, c, :, 0:DH],
            ps[:, 0:HI].rearrange("p (h e) -> p h e", h=HH))

    # ---- warmup: junk matmuls lift the PE p-state clock gate during the
    # input DMA; a junk exp pulls the ACT table load off the critical path.
    warm = const.tile([128, 512], BF)
    nc.vector.memset(warm, 1.0)
    wps = ps_f.tile([128, 512], F32, tag="f", name="wps")
    for i in range(14):
        nc.tensor.matmul(wps, warm[:, 0:128], warm, start=(i == 0),
                         stop=(i == 13))
    warm_out = const.tile([1, 32], BF)
    nc.scalar.activation(warm_out, wps[0:1, 0:32], EXP, bias=0.0, scale=0.0)

    # presweep projections (DMA-gated): K for head pair 0, first q blocks
    for b in range(NB):
        proj_kT(0, b)
    proj_qT(0, 0)
    proj_qT(1, 0)

    def norm_recip():
        # 1/denominator for both heads; the denominator rows were copied
        # into r2 rows 0 and 32 straight from PSUM at block end (quadrant-
        # aligned so one iterative reciprocal serves both heads, and so the
        # two broadcast matmuls get distinct PE row quadrants)
        rr = norm_sb.tile([33, 512], F32, tag="rr", name="rr")
        nc.vector.reciprocal(rr, r2)
        rrb = norm_sb.tile([33, 512], BF, tag="rrb", name="rrb")
        nc.vector.tensor_copy(rrb, rr)
        return rrb

    def norm_apply(p, qb, sA, sB, rrb):
        # out_allT[head rows] = staged out' * (1/denominator); the PE part
        # (broadcast matmuls) only lands here, after rrb is long done.
        bc = ps_f.tile([128, 512], F32, tag="f", name="bc")
        nc.tensor.matmul(bc[0:64, :], ones33[0:1, 0:64], rrb[0:1, :],
                         start=True, stop=True)
        nc.tensor.matmul(bc[64:128, :], ones33[32:33, 0:64], rrb[32:33, :],
                         start=True, stop=True)
        bc_sA = norm_sb.tile([64, 512], F32, tag="bcsA", name="bc_sA")
        bc_sB = norm_sb.tile([64, 512], F32, tag="bcsB", name="bc_sB")
        nc.vector.tensor_copy(bc_sA, bc[0:64, :])
        nc.vector.tensor_copy(bc_sB, bc[64:128, :])
        nc.vector.tensor_mul(
            out_allT[0:64, p, qb * 512:(qb + 1) * 512], sA, bc_sA)
        nc.vector.tensor_mul(
            out_allT[64:128, p, qb * 512:(qb + 1) * 512], sB, bc_sB)

    def outproj_t(j, t, tail=False):
        # out-proj contribution of head pair j alone (summed on the host):
        # decouples these PE fills from the other head pair's normalize
        of = out_sb.tile([128, D_OUT], BF, tag="of", name="of")
        for f in range(D_OUT // 512):
            po = ps_f.tile([128, 512], F32, tag="f", name=f"po{j}{t}{f}")
            nc.tensor.matmul(
                po, out_allT[:, j, t * 128:(t + 1) * 128],
                wo_s[:, j, f * 512:(f + 1) * 512], start=True, stop=True)
            # in the tail the idle Scalar engine takes half the casts so the
            # DVE is not the serial bottleneck of the last tiles
            if tail and f == 1:
                nc.scalar.copy(of[:, f * 512:(f + 1) * 512], po)
            else:
                nc.vector.tensor_copy(of[:, f * 512:(f + 1) * 512], po)
        nc.sync.dma_start(out[j][t * 128:(t + 1) * 128, :], of)

    def K(j, b):
        return lambda: proj_kT(j, b)

    def Q(j, b):
        return lambda: proj_qT(j, b)

    def O(j, t):
        return lambda: outproj_t(j, t)

    # per-block fill plan: "early" pops at c=3,5,7,9, "late" at c=15,17.
    # Sized so each block's PE work stays just above the 17.7us of exp the
    # ACT engine needs per block. O(j, t) is legal one block after head
    # pair j's normalize for t's query block popped (muls land ~c13).
    FILLS = {
        0: ([Q(0, 1)], []),
        1: ([Q(0, 2), K(1, 0)], [O(0, 0), O(0, 1)]),
        2: ([Q(0, 3), K(1, 1), K(1, 2)], [O(0, 2), O(0, 3)]),
        3: ([K(1, 3), Q(1, 0), Q(1, 1)], [O(0, 4), O(0, 5)]),
        4: ([Q(1, 2), Q(1, 3), O(0, 6)], [O(0, 7), O(0, 8)]),
        5: ([O(0, 9), O(0, 10), O(0, 11)], [O(1, 0), O(1, 1)]),
        6: ([O(0, 12), O(0, 13), O(0, 14), O(0, 15)], [O(1, 2), O(1, 3)]),
        7: ([O(1, 4), O(1, 5), O(1, 6), O(1, 7)], [O(1, 8), O(1, 9)]),
    }

    # attention sweep: query-block outer, head-pair inner. AV matmuls run
    # two chunks behind the score matmuls so their exp-dependency waits are
    # pre-satisfied. Accumulators are staged to SBUF at block end so the
    # next block's accumulators get PSUM slots quickly; the normalize runs
    # from staging, split across the following block.
    pending = []        # (p, qb, sA, sB, rrb) through the two norm stages
    at_l = [None] * 4
    NBLK = NQB * EC
    for bi in range(NBLK):
        p, qb = bi // NQB, bi % NQB
        hA, hB = 2 * p, 2 * p + 1
        early, late = FILLS[bi]
        early = list(early)
        late = list(late)
        oA = ps_o.tile([DH + 1, 512], F32, tag="o", name=f"oA{bi}")
        oB = ps_o.tile([DH + 1, 512], F32, tag="o", name=f"oB{bi}")
        for c in range(KC + 2):
            cc = c - 2
            if cc >= 0:
                nc.tensor.matmul(oA, vp_s[:, cc, hA, :],
                                 at_l[cc % 4][:, 0:512],
                                 start=(cc == 0), stop=(cc == KC - 1))
                nc.tensor.matmul(oB, vp_s[:, cc, hB, :],
                                 at_l[cc % 4][:, 512:1024],
                                 start=(cc == 0), stop=(cc == KC - 1))
            if c < KC:
                sT = ps_st.tile([128, 1024], F32, tag="st", name="sT")
                nc.tensor.matmul(
                    sT[:, 0:512],
                    kT_s[0:64, p, c * 128:(c + 1) * 128],
                    qT_s[0:64, p, qb * 512:(qb + 1) * 512],
                    start=True, stop=True)
                nc.tensor.matmul(
                    sT[:, 512:1024],
                    kT_s[64:128, p, c * 128:(c + 1) * 128],
                    qT_s[64:128, p, qb * 512:(qb + 1) * 512],
                    start=True, stop=True)
                at = attn_sb.tile([128, 1024], BF, tag="at", name="at")
                at_l[c % 4] = at
                nc.scalar.activation(at, sT, EXP,
                                     bias=maskb_s[:, c:c + 1], scale=SCALE)
                if bi == 0:
                    proj_v(c)
            if c == 1 and pending:
                pending[0] = pending[0][:4] + (norm_recip(),)
            if c == 11 and pending:
                pp, pqb, sA, sB, rrb = pending.pop(0)
                norm_apply(pp, pqb, sA, sB, rrb)
            if c >= 3 and c % 2 == 1 and c < 11 and early:
                early.pop(0)()
            if c >= 15 and c % 2 == 1 and late:
                late.pop(0)()
        # denominator rows straight from PSUM so the reciprocal chain is
        # not serialized behind the (bigger) stage copies
        if bi == NBLK - 1:
            nc.vector.tensor_copy(td[0:1, :], oA[DH:DH + 1, :])
            nc.vector.tensor_copy(td[32:33, :], oB[DH:DH + 1, :])
        else:
            nc.vector.tensor_copy(r2[0:1, :], oA[DH:DH + 1, :])
            nc.vector.tensor_copy(r2[32:33, :], oB[DH:DH + 1, :])
        sA = stage_sb.tile([DH, 512], F32, tag="sA", name="sA")
        sB = stage_sb.tile([DH, 512], F32, tag="sB", name="sB")
        nc.vector.tensor_copy(sA, oA[0:DH, :])
        nc.vector.tensor_copy(sB, oB[0:DH, :])
        pending.append((p, qb, sA, sB, None))

    # ---- tail: last block's normalize (via the 32x32 stream-transpose fast
    # reciprocal: ~2us instead of ~4), then the final output tiles
    pp, pqb, sA, sB, _ = pending.pop(0)
    tS = norm_sb.tile([64, 512], F32, tag="tS", name="tS")
    nc.vector.transpose(tS, td)
    nc.vector.reciprocal(
        tR.rearrange("p (i j) -> p i j", j=32)[:, :, 0:1],
        tS.rearrange("p (i j) -> p i j", j=32)[:, :, 0:1])
    tB = norm_sb.tile([64, 512], F32, tag="tB", name="tB")
    nc.vector.transpose(tB, tR)
    rrb = norm_sb.tile([33, 512], BF, tag="rrb", name="rrbt")
    nc.vector.tensor_copy(rrb[0:1, :], tB[0:1, :])
    nc.vector.tensor_copy(rrb[32:33, :], tB[32:33, :])
    norm_apply(pp, pqb, sA, sB, rrb)
    for t in range(10, 16):
        outproj_t(1, t, tail=True)


def _build():
    nc = bacc.Bacc("TRN2", target_bir_lowering=False, debug=False,
                   num_devices=NCORES)
    aps = dict(
        xT=nc.dram_tensor("xT", [NB, DC, 128, 512], BF,
                          kind="ExternalInput").ap(),
        wqT=nc.dram_tensor("wqT", [128, DC, HI], BF, kind="ExternalInput").ap(),
        wkT=nc.dram_tensor("wkT", [128, DC, HI], BF, kind="ExternalInput").ap(),
        wvT=nc.dram_tensor("wvT", [128, DC, HI], BF, kind="ExternalInput").ap(),
        woT=nc.dram_tensor("woT", [128, EC, D_OUT], BF,
                           kind="ExternalInput").ap(),
        maskb=nc.dram_tensor("maskb", [128, KC], F32, kind="ExternalInput").ap(),
        out=nc.dram_tensor("out", [EC, N, D_OUT], BF,
                           kind="ExternalOutput").ap(),
    )
    with tile.TileContext(nc) as tc:
        with ExitStack() as ctx:
            _emit(ctx, tc, **aps)
    nc.compile()
    return nc


_prog = None


def _get_prog():
    global _prog
    if _prog is None:
        _prog = _build()
    return _prog


def _make_in_maps(x, Wq, Wk, Wv, Wo, bo, mask):
    bf = ml_dtypes.bfloat16
    f32 = np.float32

    def wlayout(w):
        # [256 out, in] -> partition-major [128, in//128, 256]
        t = np.asarray(w).T.astype(bf).reshape(-1, 128, w.shape[0])
        return np.ascontiguousarray(t.transpose(1, 0, 2))

    in_maps = []
    for c in range(NCORES):
        l, hh = c // 2, c % 2
        sl = slice(hh * HI, (hh + 1) * HI)
        xTl = np.ascontiguousarray(
            x[l].T.astype(bf).reshape(DC, 128, NB, 512).transpose(2, 0, 1, 3))
        mb = np.where(mask[l], 0.0, MASK_NEG).astype(f32)
        mb = np.ascontiguousarray(mb.reshape(KC, 128).T)
        woT = np.ascontiguousarray(
            Wo[:, sl].T.astype(bf).reshape(EC, 128, D_OUT).transpose(1, 0, 2))
        in_maps.append(dict(xT=xTl, wqT=wlayout(Wq[sl]), wkT=wlayout(Wk[sl]),
                            wvT=wlayout(Wv[sl]), woT=woT, maskb=mb))
    return in_maps


def run(x, Wq, Wk, Wv, Wo, bo, mask, trace=False, tmpdir=None):
    nc = _get_prog()
    in_maps = _make_in_maps(x, Wq, Wk, Wv, Wo, bo, mask)
    res = run_bass_kernel_spmd(nc, in_maps, core_ids=list(range(NCORES)),
                               trace=trace, tmpdir=tmpdir)
    out = np.empty((L, N, D_OUT), np.float32)
    bo_f = np.asarray(bo, np.float32)
    for l in range(L):
        a = res.results[2 * l]["out"].astype(np.float32)
        b = res.results[2 * l + 1]["out"].astype(np.float32)
        out[l] = a[0] + a[1] + b[0] + b[1] + bo_f
    return out, res


def kernel(x, Wq, Wk, Wv, Wo, bo, mask):
    out, _ = run(np.asarray(x, np.float32), np.asarray(Wq, np.float32),
                 np.asarray(Wk, np.float32), np.asarray(Wv, np.float32),
                 np.asarray(Wo, np.float32), np.asarray(bo, np.float32),
                 np.asarray(mask))
    return out


# revision 28
# speedup vs baseline: 1.2777x; 1.0233x over previous
"""Cross-attention kernel for 8 Trainium2 NeuronCores (Bass/Tile).

Sharding: (L, head-half) tensor parallel. Core c handles batch l = c//2 and
heads [4*(c%2), 4*(c%2)+4) for ALL 2048 queries. Each core projects Q/K/V
only for its 4 heads (no duplicated projection work) and emits a PARTIAL
output y_part = attn_out @ Wo[:, head-block]^T; the host sums the two
partials of each pair and adds the bias during unshard.

Per-core dataflow (matmuls in bf16 with f32 PSUM accumulation):
  qT[e, n], kT[e, n]      transposed projections (feature dim on partitions)
  v'[nk, h, 65]           v projection + a ones column per head (softmax denom)
  sT[nk, nq] = kT.T @ qT  per head -- scores transposed, keys on partitions;
                          head pairs run row-packed on the PE
  attnT = exp(SCALE*sT + maskbias[nk])  one ACT op fuses scale+mask+exp+cast
  out'[65, nq] = v'.T @ attnT   rows 0..63: head out^T, row 64: denominator
  normalize split in two: the DVE reciprocal chain pops early in the next
    block; the PE broadcast matmuls + DVE muls pop mid-next-block, so the
    in-order PE queue never head-of-line blocks on the slow reciprocal
  y_part = out_allT.T @ WoT     final projection (no bias; host adds it)
"""

import numpy as np
import ml_dtypes
from contextlib import ExitStack

import concourse.bass as bass
import concourse.tile as tile
from concourse import bacc, mybir
from concourse.bass_utils import run_bass_kernel_spmd

L, N, D_IN = 4, 2048, 1024
H, DH = 8, 64
INNER = H * DH          # 512
D_OUT = D_IN
SCALE = DH ** -0.5      # 0.125
NCORES = 8
HH = H // 2             # 4 heads per core
HI = HH * DH            # 256 inner features per core
DC = D_IN // 128        # 8 contraction chunks for the projections
EC = HI // 128          # 2 feature chunks (= head pairs) per core
KC = N // 128           # 16 key chunks
NB = N // 512           # 4 key/query 512-blocks
NQB = NB                # 4 query blocks per core (all 2048 queries)
MASK_NEG = -50.0

BF = mybir.dt.bfloat16
F32 = mybir.dt.float32
EXP = mybir.ActivationFunctionType.Exp


def _emit(ctx, tc, xT, wqT, wkT, wvT, woT, maskb, out):
    nc = tc.nc

    const = ctx.enter_context(tc.tile_pool(name="const", bufs=1))
    big = ctx.enter_context(tc.tile_pool(name="big", bufs=1))
    attn_sb = ctx.enter_context(tc.tile_pool(name="attn_sb", bufs=4))
    norm_sb = ctx.enter_context(tc.tile_pool(name="norm_sb", bufs=2))
    stage_sb = ctx.enter_context(tc.tile_pool(name="stage_sb", bufs=4))
    out_sb = ctx.enter_context(tc.tile_pool(name="out_sb", bufs=4))
    ps_st = ctx.enter_context(tc.tile_pool(name="ps_st", bufs=2, space="PSUM"))
    ps_o = ctx.enter_context(tc.tile_pool(name="ps_o", bufs=2, space="PSUM"))
    ps_f = ctx.enter_context(tc.tile_pool(name="ps_f", bufs=2, space="PSUM"))

    # ---- inputs -> SBUF. Weights are partition-major (one fat DMA each);
    # x arrives as [b-block, d-chunk, 128, 512] so every (b, d) tile is one
    # contiguous 128KB transfer. Order matters: the presweep K projection
    # needs wk + xT(b) ASAP, the first attention block needs wv + maskb.
    wk_s = const.tile([128, DC, HI], BF)
    wq_s = const.tile([128, DC, HI], BF)
    wv_s = const.tile([128, DC, HI], BF)
    wo_s = const.tile([128, EC, D_OUT], BF)
    maskb_s = const.tile([128, KC], F32)
    xT_s = big.tile([128, DC, N], BF)
    nc.sync.dma_start(wk_s, wkT)
    nc.sync.dma_start(xT_s[:, :, 0:512], xT[0].rearrange("d p c -> p d c"))
    nc.sync.dma_start(wv_s, wvT)
    nc.sync.dma_start(maskb_s, maskb)
    nc.sync.dma_start(wq_s, wqT)
    for b in range(1, NB):
        nc.sync.dma_start(xT_s[:, :, b * 512:(b + 1) * 512],
                          xT[b].rearrange("d p c -> p d c"))
    nc.sync.dma_start(wo_s, woT)

    ones33 = const.tile([33, 128], BF)
    nc.vector.memset(ones33, 1.0)

    kT_s = big.tile([128, EC, N], BF)
    qT_s = big.tile([128, EC, N], BF)
    vp_s = big.tile([128, KC, HH, DH + 1], BF)
    nc.vector.memset(vp_s[:, :, :, DH], 1.0)
    out_allT = big.tile([128, EC, N], BF)
    # persistent reciprocal input; rows other than 0 and 32 stay 1.0 forever
    r2 = big.tile([33, 512], F32)
    nc.vector.memset(r2, 1.0)
    # tail fast-reciprocal scratch: 32x32 stream-transpose spreads the two
    # denominator rows across partitions so the iterative reciprocal runs on
    # 32 elements per lane instead of 512 (rows/cols never touched stay 1.0)
    td = big.tile([64, 512], F32)
    nc.vector.memset(td, 1.0)
    tR = big.tile([64, 512], F32)
    nc.vector.memset(tR, 1.0)

    def proj_kT(j, b):
        ps = ps_f.tile([128, 512], F32, tag="f", name=f"ps_k{j}{b}")
        for d in range(DC):
            nc.tensor.matmul(
                ps, wk_s[:, d, j * 128:(j + 1) * 128],
                xT_s[:, d, b * 512:(b + 1) * 512],
                start=(d == 0), stop=(d == DC - 1))
        nc.vector.tensor_copy(kT_s[:, j, b * 512:(b + 1) * 512], ps)

    def proj_qT(j, b):
        ps = ps_f.tile([128, 512], F32, tag="f", name=f"ps_q{j}{b}")
        for d in range(DC):
            nc.tensor.matmul(
                ps, wq_s[:, d, j * 128:(j + 1) * 128],
                xT_s[:, d, b * 512:(b + 1) * 512],
                start=(d == 0), stop=(d == DC - 1))
        nc.vector.tensor_copy(qT_s[:, j, b * 512:(b + 1) * 512], ps)

    def proj_v(c):
        ps = ps_f.tile([128, 512], F32, tag="f", name=f"ps_v{c}")
        for d in range(DC):
            nc.tensor.matmul(
                ps[:, 0:HI], xT_s[:, d, c * 128:(c + 1) * 128], wv_s[:, d, :],
                start=(d == 0), stop=(d == DC - 1))
        nc.vector.tensor_copy(
            vp_s[:, c, :, 0:DH],
            ps[:, 0:HI].rearrange("p (h e) -> p h e", h=HH))

    # ---- warmup: junk matmuls lift the PE p-state clock gate during the
    # input DMA; a junk exp pulls the ACT table load off the critical path.
    warm = const.tile([128, 512], BF)
    nc.vector.memset(warm, 1.0)
    wps = ps_f.tile([128, 512], F32, tag="f", name="wps")
    for i in range(14):
        nc.tensor.matmul(wps, warm[:, 0:128], warm, start=(i == 0),
                         stop=(i == 13))
    warm_out = const.tile([1, 32], BF)
    nc.scalar.activation(warm_out, wps[0:1, 0:32], EXP, bias=0.0, scale=0.0)

    # presweep projections (DMA-gated): K for head pair 0, first q blocks
    for b in range(NB):
        proj_kT(0, b)
    proj_qT(0, 0)
    proj_qT(1, 0)

    def norm_recip():
        # 1/denominator for both heads; the denominator rows were copied
        # into r2 rows 0 and 32 straight from PSUM at block end (quadrant-
        # aligned so one iterative reciprocal serves both heads, and so the
        # two broadcast matmuls get distinct PE row quadrants)
        rr = norm_sb.tile([33, 512], F32, tag="rr", name="rr")
        nc.vector.reciprocal(rr, r2)
        rrb = norm_sb.tile([33, 512], BF, tag="rrb", name="rrb")
        nc.vector.tensor_copy(rrb, rr)
        return rrb

    def norm_apply(p, qb, sA, sB, rrb):
        # out_allT[head rows] = staged out' * (1/denominator); the PE part
        # (broadcast matmuls) only lands here, after rrb is long done.
        bc = ps_f.tile([128, 512], F32, tag="f", name="bc")
        nc.tensor.matmul(bc[0:64, :], ones33[0:1, 0:64], rrb[0:1, :],
                         start=True, stop=True)
        nc.tensor.matmul(bc[64:128, :], ones33[32:33, 0:64], rrb[32:33, :],
                         start=True, stop=True)
        # muls read the broadcast tile straight from PSUM (the equal-base-
        # partition constraint only applies when both inputs are in SBUF)
        nc.vector.tensor_mul(
            out_allT[0:64, p, qb * 512:(qb + 1) * 512], sA, bc[0:64, :])
        nc.vector.tensor_mul(
            out_allT[64:128, p, qb * 512:(qb + 1) * 512], sB, bc[64:128, :])

    def outproj_t(j, t, tail=False):
        # out-proj contribution of head pair j alone (summed on the host):
        # decouples these PE fills from the other head pair's normalize
        of = out_sb.tile([128, D_OUT], BF, tag="of", name="of")
        for f in range(D_OUT // 512):
            # tail po's borrow the score-PSUM banks (idle once the sweep is
            # done) so four projections can be in flight instead of two
            pool = ps_st if tail and f == 1 else ps_f
            tg = "st" if tail and f == 1 else "f"
            po = pool.tile([128, 512], F32, tag=tg, name=f"po{j}{t}{f}")
            nc.tensor.matmul(
                po, out_allT[:, j, t * 128:(t + 1) * 128],
                wo_s[:, j, f * 512:(f + 1) * 512], start=True, stop=True)
            # in the tail the idle Scalar engine takes half the casts so the
            # DVE is not the serial bottleneck of the last tiles
            if tail and f == 1:
                nc.scalar.copy(of[:, f * 512:(f + 1) * 512], po)
            else:
                nc.vector.tensor_copy(of[:, f * 512:(f + 1) * 512], po)
        nc.sync.dma_start(out[j][t * 128:(t + 1) * 128, :], of)

    def K(j, b):
        return lambda: proj_kT(j, b)

    def Q(j, b):
        return lambda: proj_qT(j, b)

    def O(j, t):
        return lambda: outproj_t(j, t)

    # per-block fill plan: "early" pops at c=3,5,7,9, "late" at c=15,17.
    # Sized so each block's PE work stays just above the 17.7us of exp the
    # ACT engine needs per block. O(j, t) is legal one block after head
    # pair j's normalize for t's query block popped (muls land ~c13).
    FILLS = {
        0: ([Q(0, 1)], []),
        1: ([Q(0, 2), K(1, 0)], [O(0, 0), O(0, 1)]),
        2: ([Q(0, 3), K(1, 1), K(1, 2)], [O(0, 2), O(0, 3)]),
        3: ([K(1, 3), Q(1, 0), Q(1, 1)], [O(0, 4), O(0, 5)]),
        4: ([Q(1, 2), Q(1, 3), O(0, 6)], [O(0, 7), O(0, 8)]),
        5: ([O(0, 9), O(0, 10), O(0, 11)], [O(1, 0), O(1, 1)]),
        6: ([O(0, 12), O(0, 13), O(0, 14), O(0, 15)], [O(1, 2), O(1, 3)]),
        7: ([O(1, 4), O(1, 5), O(1, 6), O(1, 7)], [O(1, 8), O(1, 9)]),
    }

    # attention sweep: query-block outer, head-pair inner. AV matmuls run
    # two chunks behind the score matmuls so their exp-dependency waits are
    # pre-satisfied. Accumulators are staged to SBUF at block end so the
    # next block's accumulators get PSUM slots quickly; the normalize runs
    # from staging, split across the following block.
    pending = []        # (p, qb, sA, sB, rrb) through the two norm stages
    at_l = [None] * 4
    NBLK = NQB * EC
    for bi in range(NBLK):
        p, qb = bi // NQB, bi % NQB
        hA, hB = 2 * p, 2 * p + 1
        early, late = FILLS[bi]
        early = list(early)
        late = list(late)
        oA = ps_o.tile([DH + 1, 512], F32, tag="o", name=f"oA{bi}")
        oB = ps_o.tile([DH + 1, 512], F32, tag="o", name=f"oB{bi}")
        for c in range(KC + 2):
            cc = c - 2
            if cc >= 0:
                nc.tensor.matmul(oA, vp_s[:, cc, hA, :],
                                 at_l[cc % 4][:, 0:512],
                                 start=(cc == 0), stop=(cc == KC - 1))
                nc.tensor.matmul(oB, vp_s[:, cc, hB, :],
                                 at_l[cc % 4][:, 512:1024],
                                 start=(cc == 0), stop=(cc == KC - 1))
            if c < KC:
                sT = ps_st.tile([128, 1024], F32, tag="st", name="sT")
                nc.tensor.matmul(
                    sT[:, 0:512],
                    kT_s[0:64, p, c * 128:(c + 1) * 128],
                    qT_s[0:64, p, qb * 512:(qb + 1) * 512],
                    start=True, stop=True)
                nc.tensor.matmul(
                    sT[:, 512:1024],
                    kT_s[64:128, p, c * 128:(c + 1) * 128],
                    qT_s[64:128, p, qb * 512:(qb + 1) * 512],
                    start=True, stop=True)
                at = attn_sb.tile([128, 1024], BF, tag="at", name="at")
                at_l[c % 4] = at
                nc.scalar.activation(at, sT, EXP,
                                     bias=maskb_s[:, c:c + 1], scale=SCALE)
                if bi == 0:
                    proj_v(c)
            if c == 1 and pending:
                pending[0] = pending[0][:4] + (norm_recip(),)
            if c == 11 and pending:
                pp, pqb, sA, sB, rrb = pending.pop(0)
                norm_apply(pp, pqb, sA, sB, rrb)
            if c >= 3 and c % 2 == 1 and c < 11 and early:
                early.pop(0)()
            if c >= 15 and c % 2 == 1 and late:
                late.pop(0)()
        # denominator rows straight from PSUM so the reciprocal chain is
        # not serialized behind the (bigger) stage copies
        if bi == NBLK - 1:
            nc.vector.tensor_copy(td[0:1, :], oA[DH:DH + 1, :])
            nc.vector.tensor_copy(td[32:33, :], oB[DH:DH + 1, :])
        else:
            nc.vector.tensor_copy(r2[0:1, :], oA[DH:DH + 1, :])
            nc.vector.tensor_copy(r2[32:33, :], oB[DH:DH + 1, :])
        sA = stage_sb.tile([DH, 512], F32, tag="sA", name="sA")
        sB = stage_sb.tile([DH, 512], F32, tag="sB", name="sB")
        nc.vector.tensor_copy(sA, oA[0:DH, :])
        nc.vector.tensor_copy(sB, oB[0:DH, :])
        pending.append((p, qb, sA, sB, None))

    # ---- tail: last block's normalize (via the 32x32 stream-transpose fast
    # reciprocal: ~2us instead of ~4), then the final output tiles
    pp, pqb, sA, sB, _ = pending.pop(0)
    tS = norm_sb.tile([64, 512], F32, tag="tS", name="tS")
    nc.vector.transpose(tS, td)
    nc.vector.reciprocal(
        tR.rearrange("p (i j) -> p i j", j=32)[:, :, 0:1],
        tS.rearrange("p (i j) -> p i j", j=32)[:, :, 0:1])
    tB = norm_sb.tile([64, 512], F32, tag="tB", name="tB")
    nc.vector.transpose(tB, tR)
    rrb = norm_sb.tile([33, 512], BF, tag="rrb", name="rrbt")
    nc.vector.tensor_copy(rrb[0:1, :], tB[0:1, :])
    nc.vector.tensor_copy(rrb[32:33, :], tB[32:33, :])
    norm_apply(pp, pqb, sA, sB, rrb)
    for t in range(10, 16):
        outproj_t(1, t, tail=True)


def _build():
    nc = bacc.Bacc("TRN2", target_bir_lowering=False, debug=False,
                   num_devices=NCORES)
    aps = dict(
        xT=nc.dram_tensor("xT", [NB, DC, 128, 512], BF,
                          kind="ExternalInput").ap(),
        wqT=nc.dram_tensor("wqT", [128, DC, HI], BF, kind="ExternalInput").ap(),
        wkT=nc.dram_tensor("wkT", [128, DC, HI], BF, kind="ExternalInput").ap(),
        wvT=nc.dram_tensor("wvT", [128, DC, HI], BF, kind="ExternalInput").ap(),
        woT=nc.dram_tensor("woT", [128, EC, D_OUT], BF,
                           kind="ExternalInput").ap(),
        maskb=nc.dram_tensor("maskb", [128, KC], F32, kind="ExternalInput").ap(),
        out=nc.dram_tensor("out", [EC, N, D_OUT], BF,
                           kind="ExternalOutput").ap(),
    )
    with tile.TileContext(nc) as tc:
        with ExitStack() as ctx:
            _emit(ctx, tc, **aps)
    nc.compile()
    return nc


_prog = None


def _get_prog():
    global _prog
    if _prog is None:
        _prog = _build()
    return _prog


def _make_in_maps(x, Wq, Wk, Wv, Wo, bo, mask):
    bf = ml_dtypes.bfloat16
    f32 = np.float32

    def wlayout(w):
        # [256 out, in] -> partition-major [128, in//128, 256]
        t = np.asarray(w).T.astype(bf).reshape(-1, 128, w.shape[0])
        return np.ascontiguousarray(t.transpose(1, 0, 2))

    in_maps = []
    for c in range(NCORES):
        l, hh = c // 2, c % 2
        sl = slice(hh * HI, (hh + 1) * HI)
        xTl = np.ascontiguousarray(
            x[l].T.astype(bf).reshape(DC, 128, NB, 512).transpose(2, 0, 1, 3))
        mb = np.where(mask[l], 0.0, MASK_NEG).astype(f32)
        mb = np.ascontiguousarray(mb.reshape(KC, 128).T)
        woT = np.ascontiguousarray(
            Wo[:, sl].T.astype(bf).reshape(EC, 128, D_OUT).transpose(1, 0, 2))
        in_maps.append(dict(xT=xTl, wqT=wlayout(Wq[sl]), wkT=wlayout(Wk[sl]),
                            wvT=wlayout(Wv[sl]), woT=woT, maskb=mb))
    return in_maps


def run(x, Wq, Wk, Wv, Wo, bo, mask, trace=False, tmpdir=None):
    nc = _get_prog()
    in_maps = _make_in_maps(x, Wq, Wk, Wv, Wo, bo, mask)
    res = run_bass_kernel_spmd(nc, in_maps, core_ids=list(range(NCORES)),
                               trace=trace, tmpdir=tmpdir)
    out = np.empty((L, N, D_OUT), np.float32)
    bo_f = np.asarray(bo, np.float32)
    for l in range(L):
        a = res.results[2 * l]["out"].astype(np.float32)
        b = res.results[2 * l + 1]["out"].astype(np.float32)
        out[l] = a[0] + a[1] + b[0] + b[1] + bo_f
    return out, res


def kernel(x, Wq, Wk, Wv, Wo, bo, mask):
    out, _ = run(np.asarray(x, np.float32), np.asarray(Wq, np.float32),
                 np.asarray(Wk, np.float32), np.asarray(Wv, np.float32),
                 np.asarray(Wo, np.float32), np.asarray(bo, np.float32),
                 np.asarray(mask))
    return out


# revision 30
# speedup vs baseline: 1.2968x; 1.0150x over previous
"""Cross-attention kernel for 8 Trainium2 NeuronCores (Bass/Tile).

Sharding: (L, head-half) tensor parallel. Core c handles batch l = c//2 and
heads [4*(c%2), 4*(c%2)+4) for ALL 2048 queries. Each core projects Q/K/V
only for its 4 heads (no duplicated projection work) and emits a PARTIAL
output y_part = attn_out @ Wo[:, head-block]^T; the host sums the two
partials of each pair and adds the bias during unshard.

Per-core dataflow (matmuls in bf16 with f32 PSUM accumulation):
  qT[e, n], kT[e, n]      transposed projections (feature dim on partitions)
  v'[nk, h, 65]           v projection + a ones column per head (softmax denom)
  sT[nk, nq] = kT.T @ qT  per head -- scores transposed, keys on partitions;
                          head pairs run row-packed on the PE
  attnT = exp(SCALE*sT + maskbias[nk])  one ACT op fuses scale+mask+exp+cast
  out'[65, nq] = v'.T @ attnT   rows 0..63: head out^T, row 64: denominator
  normalize split in two: the DVE reciprocal chain pops early in the next
    block; the PE broadcast matmuls + DVE muls pop mid-next-block, so the
    in-order PE queue never head-of-line blocks on the slow reciprocal
  y_part = out_allT.T @ WoT     final projection (no bias; host adds it)
"""

import numpy as np
import ml_dtypes
from contextlib import ExitStack

import concourse.bass as bass
import concourse.tile as tile
from concourse import bacc, mybir
from concourse.bass_utils import run_bass_kernel_spmd

L, N, D_IN = 4, 2048, 1024
H, DH = 8, 64
INNER = H * DH          # 512
D_OUT = D_IN
SCALE = DH ** -0.5      # 0.125
NCORES = 8
HH = H // 2             # 4 heads per core
HI = HH * DH            # 256 inner features per core
DC = D_IN // 128        # 8 contraction chunks for the projections
EC = HI // 128          # 2 feature chunks (= head pairs) per core
KC = N // 128           # 16 key chunks
NB = N // 512           # 4 key/query 512-blocks
NQB = NB                # 4 query blocks per core (all 2048 queries)
MASK_NEG = -50.0

BF = mybir.dt.bfloat16
F32 = mybir.dt.float32
EXP = mybir.ActivationFunctionType.Exp


def _emit(ctx, tc, xT, wqT, wkT, wvT, woT, maskb, out):
    nc = tc.nc

    const = ctx.enter_context(tc.tile_pool(name="const", bufs=1))
    big = ctx.enter_context(tc.tile_pool(name="big", bufs=1))
    attn_sb = ctx.enter_context(tc.tile_pool(name="attn_sb", bufs=4))
    norm_sb = ctx.enter_context(tc.tile_pool(name="norm_sb", bufs=2))
    stage_sb = ctx.enter_context(tc.tile_pool(name="stage_sb", bufs=4))
    out_sb = ctx.enter_context(tc.tile_pool(name="out_sb", bufs=4))
    ps_st = ctx.enter_context(tc.tile_pool(name="ps_st", bufs=2, space="PSUM"))
    ps_o = ctx.enter_context(tc.tile_pool(name="ps_o", bufs=2, space="PSUM"))
    ps_f = ctx.enter_context(tc.tile_pool(name="ps_f", bufs=2, space="PSUM"))

    # ---- inputs -> SBUF. Weights are partition-major (one fat DMA each);
    # x arrives as [b-block, d-chunk, 128, 512] so every (b, d) tile is one
    # contiguous 128KB transfer. Order matters: the presweep K projection
    # needs wk + xT(b) ASAP, the first attention block needs wv + maskb.
    wk_s = const.tile([128, DC, HI], BF)
    wq_s = const.tile([128, DC, HI], BF)
    wv_s = const.tile([128, DC, HI], BF)
    wo_s = const.tile([128, EC, D_OUT], BF)
    maskb_s = const.tile([128, KC], F32)
    xT_s = big.tile([128, DC, N], BF)
    nc.sync.dma_start(wk_s, wkT)
    nc.sync.dma_start(xT_s[:, :, 0:512], xT[0].rearrange("d p c -> p d c"))
    nc.sync.dma_start(wv_s, wvT)
    nc.sync.dma_start(maskb_s, maskb)
    nc.sync.dma_start(wq_s, wqT)
    for b in range(1, NB):
        nc.sync.dma_start(xT_s[:, :, b * 512:(b + 1) * 512],
                          xT[b].rearrange("d p c -> p d c"))
    nc.sync.dma_start(wo_s, woT)

    ones33 = const.tile([33, 128], BF)
    nc.vector.memset(ones33, 1.0)

    kT_s = big.tile([128, EC, N], BF)
    qT_s = big.tile([128, EC, N], BF)
    vp_s = big.tile([128, KC, HH, DH + 1], BF)
    nc.vector.memset(vp_s[:, :, :, DH], 1.0)
    out_allT = big.tile([128, EC, N], BF)
    # persistent reciprocal input; rows other than 0 and 32 stay 1.0 forever
    r2 = big.tile([33, 512], F32)
    nc.vector.memset(r2, 1.0)
    # tail fast-reciprocal scratch: 32x32 stream-transpose spreads the two
    # denominator rows across partitions so the iterative reciprocal runs on
    # 32 elements per lane instead of 512 (rows/cols never touched stay 1.0)
    td = big.tile([64, 512], F32)
    nc.vector.memset(td, 1.0)
    tR = big.tile([64, 512], F32)
    nc.vector.memset(tR, 1.0)

    def proj_kT(j, b):
        ps = ps_f.tile([128, 512], F32, tag="f", name=f"ps_k{j}{b}")
        for d in range(DC):
            nc.tensor.matmul(
                ps, wk_s[:, d, j * 128:(j + 1) * 128],
                xT_s[:, d, b * 512:(b + 1) * 512],
                start=(d == 0), stop=(d == DC - 1))
        nc.vector.tensor_copy(kT_s[:, j, b * 512:(b + 1) * 512], ps)

    def proj_qT(j, b):
        ps = ps_f.tile([128, 512], F32, tag="f", name=f"ps_q{j}{b}")
        for d in range(DC):
            nc.tensor.matmul(
                ps, wq_s[:, d, j * 128:(j + 1) * 128],
                xT_s[:, d, b * 512:(b + 1) * 512],
                start=(d == 0), stop=(d == DC - 1))
        nc.vector.tensor_copy(qT_s[:, j, b * 512:(b + 1) * 512], ps)

    def proj_v(c):
        ps = ps_f.tile([128, 512], F32, tag="f", name=f"ps_v{c}")
        for d in range(DC):
            nc.tensor.matmul(
                ps[:, 0:HI], xT_s[:, d, c * 128:(c + 1) * 128], wv_s[:, d, :],
                start=(d == 0), stop=(d == DC - 1))
        nc.vector.tensor_copy(
            vp_s[:, c, :, 0:DH],
            ps[:, 0:HI].rearrange("p (h e) -> p h e", h=HH))

    # ---- warmup: junk matmuls lift the PE p-state clock gate during the
    # input DMA; a junk exp pulls the ACT table load off the critical path.
    warm = const.tile([128, 512], BF)
    nc.vector.memset(warm, 1.0)
    wps = ps_f.tile([128, 512], F32, tag="f", name="wps")
    for i in range(14):
        nc.tensor.matmul(wps, warm[:, 0:128], warm, start=(i == 0),
                         stop=(i == 13))
    warm_out = const.tile([1, 32], BF)
    nc.scalar.activation(warm_out, wps[0:1, 0:32], EXP, bias=0.0, scale=0.0)

    # presweep projections (DMA-gated): K for head pair 0, first q blocks
    for b in range(NB):
        proj_kT(0, b)
    proj_qT(0, 0)
    proj_qT(1, 0)

    def norm_recip():
        # 1/denominator for both heads; the denominator rows were copied
        # into r2 rows 0 and 32 straight from PSUM at block end (quadrant-
        # aligned so one iterative reciprocal serves both heads, and so the
        # two broadcast matmuls get distinct PE row quadrants)
        rr = norm_sb.tile([33, 512], F32, tag="rr", name="rr")
        nc.vector.reciprocal(rr, r2)
        rrb = norm_sb.tile([33, 512], BF, tag="rrb", name="rrb")
        nc.vector.tensor_copy(rrb, rr)
        return rrb

    def norm_apply(p, qb, sA, sB, rrb):
        # out_allT[head rows] = staged out' * (1/denominator); the PE part
        # (broadcast matmuls) only lands here, after rrb is long done.
        bc = ps_f.tile([128, 512], F32, tag="f", name="bc")
        nc.tensor.matmul(bc[0:64, :], ones33[0:1, 0:64], rrb[0:1, :],
                         start=True, stop=True)
        nc.tensor.matmul(bc[64:128, :], ones33[32:33, 0:64], rrb[32:33, :],
                         start=True, stop=True)
        # muls read the broadcast tile straight from PSUM (the equal-base-
        # partition constraint only applies when both inputs are in SBUF)
        nc.vector.tensor_mul(
            out_allT[0:64, p, qb * 512:(qb + 1) * 512], sA, bc[0:64, :])
        nc.vector.tensor_mul(
            out_allT[64:128, p, qb * 512:(qb + 1) * 512], sB, bc[64:128, :])

    def outproj_t(j, t, tail=False):
        # out-proj contribution of head pair j alone (summed on the host):
        # decouples these PE fills from the other head pair's normalize
        of = out_sb.tile([128, D_OUT], BF, tag="of", name="of")
        for f in range(D_OUT // 512):
            # tail po's borrow the score-PSUM banks (idle once the sweep is
            # done) so four projections can be in flight instead of two
            pool = ps_st if tail and f == 1 else ps_f
            tg = "st" if tail and f == 1 else "f"
            po = pool.tile([128, 512], F32, tag=tg, name=f"po{j}{t}{f}")
            nc.tensor.matmul(
                po, out_allT[:, j, t * 128:(t + 1) * 128],
                wo_s[:, j, f * 512:(f + 1) * 512], start=True, stop=True)
            # in the tail the idle Scalar engine takes half the casts so the
            # DVE is not the serial bottleneck of the last tiles
            if tail and f == 1:
                nc.scalar.copy(of[:, f * 512:(f + 1) * 512], po)
            else:
                nc.vector.tensor_copy(of[:, f * 512:(f + 1) * 512], po)
        nc.sync.dma_start(out[j][t * 128:(t + 1) * 128, :], of)

    def K(j, b):
        return lambda: proj_kT(j, b)

    def Q(j, b):
        return lambda: proj_qT(j, b)

    def O(j, t):
        return lambda: outproj_t(j, t)

    # per-block fill plan: "early" pops at c=3,5,7,9, "late" at c=15,17.
    # Sized so each block's PE work stays just above the 17.7us of exp the
    # ACT engine needs per block. O(j, t) is legal one block after head
    # pair j's normalize for t's query block popped (muls land ~c13).
    FILLS = {
        0: ([Q(0, 1)], []),
        1: ([Q(0, 2), K(1, 0)], [O(0, 0), O(0, 1)]),
        2: ([Q(0, 3), K(1, 1), K(1, 2)], [O(0, 2), O(0, 3)]),
        3: ([K(1, 3), Q(1, 0), Q(1, 1)], [O(0, 4), O(0, 5)]),
        4: ([Q(1, 2), Q(1, 3), O(0, 6)], [O(0, 7), O(0, 8)]),
        5: ([O(0, 9), O(0, 10), O(0, 11)], [O(1, 0), O(1, 1)]),
        6: ([O(0, 12), O(0, 13), O(0, 14), O(0, 15)], [O(1, 2), O(1, 3)]),
        7: ([O(1, 4), O(1, 5), O(1, 6), O(1, 7)], [O(1, 8), O(1, 9)]),
    }

    # attention sweep: query-block outer, head-pair inner. AV matmuls run
    # two chunks behind the score matmuls so their exp-dependency waits are
    # pre-satisfied. Accumulators are staged to SBUF at block end so the
    # next block's accumulators get PSUM slots quickly; the normalize runs
    # from staging, split across the following block.
    pending = []        # (p, qb, sA, sB, rrb) through the two norm stages
    at_l = [None] * 4
    NBLK = NQB * EC
    for bi in range(NBLK):
        p, qb = bi // NQB, bi % NQB
        hA, hB = 2 * p, 2 * p + 1
        early, late = FILLS[bi]
        early = list(early)
        late = list(late)
        oA = ps_o.tile([DH + 1, 512], F32, tag="o", name=f"oA{bi}")
        oB = ps_o.tile([DH + 1, 512], F32, tag="o", name=f"oB{bi}")
        for c in range(KC + 2):
            cc = c - 2
            if cc >= 0:
                nc.tensor.matmul(oA, vp_s[:, cc, hA, :],
                                 at_l[cc % 4][:, 0:512],
                                 start=(cc == 0), stop=(cc == KC - 1))
                nc.tensor.matmul(oB, vp_s[:, cc, hB, :],
                                 at_l[cc % 4][:, 512:1024],
                                 start=(cc == 0), stop=(cc == KC - 1))
            if c < KC:
                sT = ps_st.tile([128, 1024], F32, tag="st", name="sT")
                nc.tensor.matmul(
                    sT[:, 0:512],
                    kT_s[0:64, p, c * 128:(c + 1) * 128],
                    qT_s[0:64, p, qb * 512:(qb + 1) * 512],
                    start=True, stop=True)
                nc.tensor.matmul(
                    sT[:, 512:1024],
                    kT_s[64:128, p, c * 128:(c + 1) * 128],
                    qT_s[64:128, p, qb * 512:(qb + 1) * 512],
                    start=True, stop=True)
                at = attn_sb.tile([128, 1024], BF, tag="at", name="at")
                at_l[c % 4] = at
                nc.scalar.activation(at, sT, EXP,
                                     bias=maskb_s[:, c:c + 1], scale=SCALE)
                if bi == 0:
                    proj_v(c)
            if c == 1 and pending:
                pending[0] = pending[0][:4] + (norm_recip(),)
            if c == 11 and pending:
                pp, pqb, sA, sB, rrb = pending.pop(0)
                norm_apply(pp, pqb, sA, sB, rrb)
            if c >= 3 and c % 2 == 1 and c < 11 and early:
                early.pop(0)()
            if c >= 15 and c % 2 == 1 and late:
                late.pop(0)()
        # denominator rows straight from PSUM, BEFORE the bigger stage
        # copies: the reciprocal chain is longer than anything that depends
        # on the stages, so it must enter the DVE queue first
        if bi == NBLK - 1:
            nc.vector.tensor_copy(td[0:1, :], oA[DH:DH + 1, :])
            nc.vector.tensor_copy(td[32:33, :], oB[DH:DH + 1, :])
            tS = norm_sb.tile([64, 512], F32, tag="tS", name="tS")
            nc.vector.transpose(tS, td)
            nc.vector.reciprocal(
                tR.rearrange("p (i j) -> p i j", j=32)[:, :, 0:1],
                tS.rearrange("p (i j) -> p i j", j=32)[:, :, 0:1])
            tB = big.tile([64, 512], F32)
            nc.vector.transpose(tB, tR)
        else:
            nc.vector.tensor_copy(r2[0:1, :], oA[DH:DH + 1, :])
            nc.vector.tensor_copy(r2[32:33, :], oB[DH:DH + 1, :])
        sA = stage_sb.tile([DH, 512], F32, tag="sA", name="sA")
        sB = stage_sb.tile([DH, 512], F32, tag="sB", name="sB")
        nc.vector.tensor_copy(sA, oA[0:DH, :])
        nc.vector.tensor_copy(sB, oB[0:DH, :])
        pending.append((p, qb, sA, sB, None))

    # ---- tail: the last normalize's reciprocal already ran (transposed,
    # ~0.25us) inside the block end; two output tiles that only depend on
    # the PREVIOUS block's normalize fill the PE while rrb is cast.
    pp, pqb, sA, sB, _ = pending.pop(0)
    rrb = norm_sb.tile([33, 512], BF, tag="rrb", name="rrbt")
    nc.vector.tensor_copy(rrb[0:1, :], tB[0:1, :])
    nc.vector.tensor_copy(rrb[32:33, :], tB[32:33, :])
    outproj_t(1, 10, tail=True)
    outproj_t(1, 11, tail=True)
    norm_apply(pp, pqb, sA, sB, rrb)
    for t in range(12, 16):
        outproj_t(1, t, tail=True)


def _build():
    nc = bacc.Bacc("TRN2", target_bir_lowering=False, debug=False,
                   num_devices=NCORES)
    aps = dict(
        xT=nc.dram_tensor("xT", [NB, DC, 128, 512], BF,
                          kind="ExternalInput").ap(),
        wqT=nc.dram_tensor("wqT", [128, DC, HI], BF, kind="ExternalInput").ap(),
        wkT=nc.dram_tensor("wkT", [128, DC, HI], BF, kind="ExternalInput").ap(),
        wvT=nc.dram_tensor("wvT", [128, DC, HI], BF, kind="ExternalInput").ap(),
        woT=nc.dram_tensor("woT", [128, EC, D_OUT], BF,
                           kind="ExternalInput").ap(),
        maskb=nc.dram_tensor("maskb", [128, KC], F32, kind="ExternalInput").ap(),
        out=nc.dram_tensor("out", [EC, N, D_OUT], BF,
                           kind="ExternalOutput").ap(),
    )
    with tile.TileContext(nc) as tc:
        with ExitStack() as ctx:
            _emit(ctx, tc, **aps)
    nc.compile()
    return nc


_prog = None


def _get_prog():
    global _prog
    if _prog is None:
        _prog = _build()
    return _prog


def _make_in_maps(x, Wq, Wk, Wv, Wo, bo, mask):
    bf = ml_dtypes.bfloat16
    f32 = np.float32

    def wlayout(w):
        # [256 out, in] -> partition-major [128, in//128, 256]
        t = np.asarray(w).T.astype(bf).reshape(-1, 128, w.shape[0])
        return np.ascontiguousarray(t.transpose(1, 0, 2))

    in_maps = []
    for c in range(NCORES):
        l, hh = c // 2, c % 2
        sl = slice(hh * HI, (hh + 1) * HI)
        xTl = np.ascontiguousarray(
            x[l].T.astype(bf).reshape(DC, 128, NB, 512).transpose(2, 0, 1, 3))
        mb = np.where(mask[l], 0.0, MASK_NEG).astype(f32)
        mb = np.ascontiguousarray(mb.reshape(KC, 128).T)
        woT = np.ascontiguousarray(
            Wo[:, sl].T.astype(bf).reshape(EC, 128, D_OUT).transpose(1, 0, 2))
        in_maps.append(dict(xT=xTl, wqT=wlayout(Wq[sl]), wkT=wlayout(Wk[sl]),
                            wvT=wlayout(Wv[sl]), woT=woT, maskb=mb))
    return in_maps


def run(x, Wq, Wk, Wv, Wo, bo, mask, trace=False, tmpdir=None):
    nc = _get_prog()
    in_maps = _make_in_maps(x, Wq, Wk, Wv, Wo, bo, mask)
    res = run_bass_kernel_spmd(nc, in_maps, core_ids=list(range(NCORES)),
                               trace=trace, tmpdir=tmpdir)
    out = np.empty((L, N, D_OUT), np.float32)
    bo_f = np.asarray(bo, np.float32)
    for l in range(L):
        a = res.results[2 * l]["out"].astype(np.float32)
        b = res.results[2 * l + 1]["out"].astype(np.float32)
        out[l] = a[0] + a[1] + b[0] + b[1] + bo_f
    return out, res


def kernel(x, Wq, Wk, Wv, Wo, bo, mask):
    out, _ = run(np.asarray(x, np.float32), np.asarray(Wq, np.float32),
                 np.asarray(Wk, np.float32), np.asarray(Wv, np.float32),
                 np.asarray(Wo, np.float32), np.asarray(bo, np.float32),
                 np.asarray(mask))
    return out
